# revision 30
# baseline (speedup 1.0000x reference)
"""Trainium2 Bass kernel for nn_CFAConv (cross-feature attention + conv block).

Self-contained: takes full unsharded inputs, shards (batch, image-half) across
8 NeuronCores, runs one SPMD Bass/Tile NEFF, and combines partial results on
the host.

Math (validated against the jax reference in numpy):
  x1 = w_q@in0 + b_q ; x2 = w_k@in0 + b_k ; x3 = w_v@in1 + b_v  (1x1 convs)
  aff = softmax_j(x2^T x3) ; x0 = x1 @ aff
  x0' = gelu(bn0(w_o@x0 + b_o))
  y = gelu(bn(conv3x3(concat(x0', in0)))) ; y = gelu(bn(conv3x3(y)))
  out = max_spatial(y + x0')
On-device simplifications:
  - softmax over j is invariant to per-column shifts => b_k drops entirely
  - x2^T(x3 + b_v) = x2^T x3 + (x2^T b_v) 1^T    => fold b_v into x3
  - (x1 + b_q 1^T) @ aff = x1@aff + b_q 1^T (aff columns sum to 1)
    => fold w_o@b_q into the out-projection bias (host-side)
  - eval-mode BN folds to per-channel scale/bias, fused into the gelu ACT op
  - softmax normalization deferred past the x1@exp(S) matmul (divide x0 by
    column sums); sums via a 5-level bf16 DVE pre-sum tree + one ones-matmul
  - no max-subtraction in softmax: |S| <= ~60 here; exp fits fp32 (max ~e88)
Precision: bf16 operands with fp32 PSUM accumulation for the attention path;
the two 3x3 convs run in fp8e4m3 with DoubleRow perf mode (2 contraction
tiles per pass at 0.5 cycles/row):
  - conv0 x0'-half: weights + acts naive fp8 (x0' is small vs in0 => cheap)
  - conv0 in0-half: weights hi+lo fp8 split, in0 hi+lo fp8 split (host-side),
    3-term product (Wh Xh + Wh Xl + Wl Xh)
  - conv1: weights hi+lo (host), c0 hi+lo split on DVE, 3-term
  (numpy bit-model: 1.3e-2 final rel err vs the 2e-2 budget)
Sharding: 8 cores = (4 batches) x (top/bottom image half). Each core computes
a 34-row window (32 owned + halo) so the two 3x3 convs need no communication;
per-row maxes [256, 34] go to the host which slices owned rows and reduces.
"""

from contextlib import ExitStack

import ml_dtypes
import numpy as np

import concourse.bass as bass
import concourse.tile as tile
from concourse import bacc, mybir
from concourse.bass_utils import run_bass_kernel_spmd

B, C, H, W = 4, 256, 64, 64
Ch = C // 2          # 128
N = H * W            # 4096
ROWS = 34            # per-core row window (32 owned + 2 halo)
KW = ROWS * W        # 2176 window positions
EPS = 1e-5

F32 = mybir.dt.float32
BF16 = mybir.dt.bfloat16
F8 = mybir.dt.float8e4
AF = mybir.ActivationFunctionType
AX = mybir.AxisListType
DR = mybir.MatmulPerfMode.DoubleRow
BF16NP = ml_dtypes.bfloat16
F8NP = ml_dtypes.float8_e4m3

# attention k-tiles over the 2176-column window
K_TILES = [(0, 512), (512, 512), (1024, 512), (1536, 512), (2048, 128)]
# conv output row-tiles (local rows 1..34 of the 36-row padded buffer)
ROW_TILES = [(1, 8), (9, 8), (17, 8), (25, 8), (33, 2)]

_CACHED = {}


def build_program():
    nc = bacc.Bacc("TRN2", target_bir_lowering=False, debug=False)

    def din(name, shape, dt=F32):
        return nc.dram_tensor(name, shape, dt, kind="ExternalInput").ap()

    in0b_d = din("in0b", [C, N], BF16)
    in1b_d = din("in1b", [C, KW], BF16)
    # in0 conv window, fp8 hi/lo, pre-padded to 66 cols (zero side columns)
    in0h_d = din("in0h", [C, ROWS * 66], F8)
    in0l_d = din("in0l", [C, ROWS * 66], F8)
    wq_t = din("wq_t", [C, Ch], BF16)     # (c, i)
    wk_t = din("wk_t", [C, Ch], BF16)
    wv_t = din("wv_t", [C, Ch], BF16)
    wo_t = din("wo_t", [Ch, C], BF16)     # (i, o)
    bv = din("bv", [Ch, 1])
    bias6_d = din("bias6", [6, C, 1])     # ao, bo, a0, b0, a1, b1
    # conv0 weights: [x0-half naive, in0-half hi, in0-half lo] (tap, ci, o)
    w0all_d = din("w0all", [3, 9, C, C], F8)
    w1all_d = din("w1all", [2, 9, C, C], F8)  # [hi, lo]
    idm_d = din("idm", [128, 128], BF16)
    out = nc.dram_tensor("out", [C, ROWS], F32, kind="ExternalOutput").ap()

    with tile.TileContext(nc) as tc, ExitStack() as ctx:
        persist = ctx.enter_context(tc.tile_pool(name="persist", bufs=1))
        psum = ctx.enter_context(tc.tile_pool(name="psum", bufs=2, space="PSUM"))
        psum1 = ctx.enter_context(tc.tile_pool(name="psum1", bufs=2, space="PSUM"))
        small = ctx.enter_context(tc.tile_pool(name="small", bufs=3))

        # ---- inputs: bf16 quarters of in0 (one DMA each: per-slice deps
        # because Tile dependencies are whole-tile). Weights + a small first
        # slice of in0 go first so the first matmul starts ASAP. ----
        wk_s = persist.tile([128, 2, Ch], BF16, tag="wk")
        nc.sync.dma_start(out=wk_s, in_=wk_t.rearrange("(a p) n -> p a n", a=2))
        wq_s = persist.tile([128, 2, Ch], BF16, tag="wq")
        nc.sync.dma_start(out=wq_s, in_=wq_t.rearrange("(a p) n -> p a n", a=2))
        in0q0a = persist.tile([128, 2, 128], BF16, tag="in0q0a")
        nc.sync.dma_start(
            out=in0q0a,
            in_=in0b_d.rearrange("(a p) n -> p a n", a=2)[:, :, 0:128])
        in0q0b = persist.tile([128, 2, 384], BF16, tag="in0q0b")
        nc.sync.dma_start(
            out=in0q0b,
            in_=in0b_d.rearrange("(a p) n -> p a n", a=2)[:, :, 128:512])
        in0q = [None] + [persist.tile([128, 2, 512], BF16, tag=f"in0q{q}",
                                      name=f"in0q{q}") for q in range(1, 8)]
        in1q = [persist.tile([128, 2, 1152], BF16, tag=f"in1q{q}",
                             name=f"in1q{q}") for q in range(2)]
        # even in0 quarters + in1q0 dispatch from the ACT HWDGE queue (idle
        # until the exps) so quarter arrival keeps pace with the projections
        for q in (2, 4, 6):
            nc.scalar.dma_start(
                out=in0q[q],
                in_=in0b_d.rearrange("(a p) n -> p a n", a=2)[
                    :, :, q * 512:(q + 1) * 512])
        nc.scalar.dma_start(
            out=in1q[0][:, :, :1024],
            in_=in1b_d.rearrange("(a p) n -> p a n", a=2)[:, :, 0:1024])
        wv_s = persist.tile([128, 2, Ch], BF16, tag="wv")
        nc.sync.dma_start(out=wv_s, in_=wv_t.rearrange("(a p) n -> p a n", a=2))
        for q in (1, 3, 5, 7):
            nc.sync.dma_start(
                out=in0q[q],
                in_=in0b_d.rearrange("(a p) n -> p a n", a=2)[
                    :, :, q * 512:(q + 1) * 512])
        bv_s = persist.tile([128, 1], F32, tag="bv")
        nc.sync.dma_start(out=bv_s, in_=bv)
        nc.sync.dma_start(
            out=in1q[1],
            in_=in1b_d.rearrange("(a p) n -> p a n", a=2)[:, :, 1024:KW])
        wo_s = persist.tile([128, C], BF16, tag="wo")
        nc.sync.dma_start(out=wo_s, in_=wo_t)
        bias_s = persist.tile([128, 12], F32, tag="bias6")
        nc.sync.dma_start(out=bias_s,
                          in_=bias6_d.rearrange("t (a p) o -> p (t a o)", a=2))
        ao_s, bo_s = bias_s[:, 0:2], bias_s[:, 2:4]
        a0_s, b0_s = bias_s[:, 4:6], bias_s[:, 6:8]
        a1_s, b1_s = bias_s[:, 8:10], bias_s[:, 10:12]
        ones_s = persist.tile([128, 1], BF16, tag="ones")
        nc.vector.memset(ones_s, 1.0)
        ones_row = persist.tile([1, 128], F32, tag="ones_row")
        nc.vector.memset(ones_row, 1.0)
        id_s = persist.tile([128, 128], BF16, tag="idm")
        nc.sync.dma_start(out=id_s, in_=idm_d)

        # ---- projections: x2 [ch, N], x1T [j, i], x3 [ch, KW] (all bf16) --
        x2_s = persist.tile([128, N], BF16, tag="x2")
        x1t_s = persist.tile([128, 32, Ch], BF16, tag="x1t")
        x3_s = persist.tile([128, KW], BF16, tag="x3")

        def jc_slices(jc, lo, hi):
            """moving-operand slices [lo:hi) of in0 quarter jc (jc 0 is split
            into a 128-col head so the first matmul follows a small DMA)."""
            if jc > 0:
                return [(in0q[jc], lo, hi, lo)]
            out = []
            if lo < 128:
                out.append((in0q0a, lo, min(hi, 128), lo))
            if hi > 128:
                out.append((in0q0b, max(lo, 128) - 128, hi - 128,
                            max(lo, 128)))
            return out

        for jc in range(8):
            # x2 chunk: one accumulation group in one PSUM bank
            ps2 = psum.tile([128, 2, 512], F32, tag="ps_S")
            mms = [(t, a, b, o, cc) for cc in range(2)
                   for (t, a, b, o) in jc_slices(jc, 0, 512)]
            for i, (t, a, b, o, cc) in enumerate(mms):
                nc.tensor.matmul(ps2[:, 0, o:o + b - a], wk_s[:, cc, :],
                                 t[:, cc, a:b],
                                 start=(i == 0), stop=(i == len(mms) - 1))
            nc.vector.tensor_copy(x2_s[:, jc * 512:(jc + 1) * 512], ps2[:, 0, :])
            # x1t: all 4 j-subchunks in ONE group/bank, evicted in one ACT copy
            ps1 = psum1.tile([128, 512], F32, tag="ps_acc")
            mms = [(js, t, a, b, cc) for js in range(4) for cc in range(2)
                   for (t, a, b, _) in jc_slices(jc, js * 128, (js + 1) * 128)]
            for i, (js, t, a, b, cc) in enumerate(mms):
                nc.tensor.matmul(
                    ps1[:, js * 128:js * 128 + Ch],
                    t[:, cc, a:b], wq_s[:, cc, :],
                    start=(i == 0), stop=(i == len(mms) - 1))
            if jc < 3:  # ACT is dispatching input DMAs early on; use DVE
                nc.vector.tensor_copy(
                    x1t_s[:, jc * 4:(jc + 1) * 4, :],
                    ps1.rearrange("p (a c) -> p a c", c=Ch))
            else:
                nc.scalar.activation(
                    x1t_s[:, jc * 4:(jc + 1) * 4, :],
                    ps1.rearrange("p (a c) -> p a c", c=Ch), AF.Copy)

        for k0, ksz in K_TILES:
            iq, off = (0, k0) if k0 < 1024 else (1, k0 - 1024)
            ps3 = psum.tile([128, 2, 512], F32, tag="ps_S")
            for cc in range(2):
                nc.tensor.matmul(ps3[:, 0, :ksz], wv_s[:, cc, :],
                                 in1q[iq][:, cc, off:off + ksz],
                                 start=(cc == 0), stop=(cc == 1))
            # x3 = psum + b_v : folds the v-bias into the affinity logits
            nc.vector.tensor_scalar_add(x3_s[:, k0:k0 + ksz], ps3[:, 0, :ksz],
                                        bv_s)

        # ---- conv buffers (fp8, padded 36x66 with zero ring) ----
        convbuf = ctx.enter_context(tc.tile_pool(name="convbuf", bufs=1))
        ybuf = convbuf.tile([128, 2, 36, 66], F8, tag="ybuf")   # x0' chunks
        in0h_s = convbuf.tile([128, 2, 36, 66], F8, tag="in0h")
        in0l_s = convbuf.tile([128, 2, 36, 66], F8, tag="in0l")
        c0h = convbuf.tile([128, 2, 36, 66], F8, tag="c0h")
        c0l = convbuf.tile([128, 2, 36, 66], F8, tag="c0l")
        c0f = convbuf.tile([128, 2, ROWS, W], BF16, tag="c0f")
        for tl in (ybuf, in0h_s, in0l_s, c0h, c0l):
            # zero the pad ring (write-only memset; reading uninitialized
            # SBUF can produce NaNs)
            nc.vector.memset(tl[:, :, 0, :], 0.0)
            nc.vector.memset(tl[:, :, 35, :], 0.0)
            if tl is in0h_s or tl is in0l_s:
                continue  # side columns arrive zero-padded via the DMA
            nc.vector.memset(tl[:, :, 1:35, 0:1], 0.0)
            nc.vector.memset(tl[:, :, 1:35, 65:66], 0.0)
        # in0 conv window ships as fp8 hi/lo straight into the padded tiles
        # (host pre-pads the 66-col side ring so the DMA stays 3-dim)
        nc.sync.dma_start(
            out=in0h_s[:, :, 1:35, :],
            in_=in0h_d.rearrange("(a p) n -> p a n", a=2))
        nc.sync.dma_start(
            out=in0l_s[:, :, 1:35, :],
            in_=in0l_d.rearrange("(a p) n -> p a n", a=2))

        # ---- conv0 weights (early: the in0-half partial sums run inside the
        # attention phase to fill PE slack while ACT grinds the exps) ----
        w0all_s = persist.tile([128, 54, C], F8, tag="w0all")
        nc.sync.dma_start(
            out=w0all_s,
            in_=w0all_d.rearrange("s t (a p) o -> p (s t a) o", a=2))
        w0x_s = w0all_s[:, 0:18]
        w0inh_s = w0all_s[:, 18:36]
        w0inl_s = w0all_s[:, 36:54]
        inpart = [persist.tile([128, 512], BF16, tag=f"inpart{g}",
                               name=f"inpart{g}") for g in range(10)]
        terms_in0 = [(w0inh_s, in0h_s), (w0inh_s, in0l_s), (w0inl_s, in0h_s)]

        def emit_in0_partial(g):
            """27 DoubleRow passes of conv0's in0-half for group g=(rt,oc),
            evicted to SBUF bf16 for later re-injection."""
            (r0, nr), oc = ROW_TILES[g // 2], g % 2
            ps = psum1.tile([128, 512], F32, tag="ps_cv", name=f"cv{g}")
            pcv = ps[:, :nr * W].rearrange("p (r w) -> p r w", w=W)
            i_mm, n_mm = 0, 27
            for w_s, x_s in terms_in0:
                for t9 in range(9):
                    dh, dw = divmod(t9, 3)
                    nc.tensor.matmul(
                        pcv,
                        w_s[:, t9 * 2:t9 * 2 + 2, oc * 128:(oc + 1) * 128],
                        x_s[:, :, r0 + dh - 1:r0 + dh - 1 + nr, dw:dw + W],
                        start=(i_mm == 0), stop=(i_mm == n_mm - 1),
                        perf_mode=DR)
                    i_mm += 1
            nc.vector.tensor_copy(inpart[g][:, :nr * W], ps[:, :nr * W])

        # ---- attention: S = x2^T x3, exp, x0 = x1 @ exp, sums, normalize ----
        attn = ctx.enter_context(tc.tile_pool(name="attn", bufs=4))
        attn2 = ctx.enter_context(tc.tile_pool(name="attn2", bufs=2))
        dram = ctx.enter_context(tc.tile_pool(name="dram", bufs=5, space="DRAM"))
        x0n_s = persist.tile([128, KW], BF16, tag="x0n")
        for kt, (k0, ksz) in enumerate(K_TILES):
            # four quarter-tiles under one bufs=4 tag: stage-2 consumes a
            # quarter while later quarters' exps still run, and the next
            # k-tile's exps begin as soon as a quarter is drained
            expS_h = [attn.tile([128, 8, 512], BF16, tag="expS",
                                name=f"expS{k0}_{h}") for h in range(4)]
            # ssum shares the ps_cv ring with the conv0 in0-half partials:
            # both have fast consumers so the rotation never stalls the PE
            ssum_t = psum1.tile([128, 512], F32, tag="ps_cv",
                                name=f"ssum{k0}")
            for mh in range(16):  # chunk pairs
                sp = psum.tile([128, 2, 512], F32, tag="ps_S")
                for i in range(2):
                    m = 2 * mh + i
                    nc.tensor.matmul(
                        sp[:, i, :ksz],
                        x2_s[:, m * 128:(m + 1) * 128],
                        x3_s[:, k0:k0 + ksz],
                        start=True, stop=True)
                eh = expS_h[mh // 4]
                nc.scalar.activation(
                    eh[:, (2 * mh) % 8:(2 * mh) % 8 + 2, :ksz],
                    sp[:, :, :ksz], AF.Exp)
            # 5-level bf16 pre-sum tree on DVE collapses the softmax
            # column-sum to ONE ones-matmul pass (sum error ~0.3%, only
            # scales the normalization)
            octs = attn2.tile([128, 4, 512], BF16, tag="oct")
            for h in range(4):
                pair = attn.tile([128, 4, 512], BF16, tag="pair",
                                 name=f"pair{k0}_{h}")
                for i in range(4):
                    nc.vector.tensor_add(pair[:, i, :ksz],
                                         expS_h[h][:, 2 * i, :ksz],
                                         expS_h[h][:, 2 * i + 1, :ksz])
                quad = attn.tile([128, 2, 512], BF16, tag="quad",
                                 name=f"quad{k0}_{h}")
                for i in range(2):
                    nc.vector.tensor_add(quad[:, i, :ksz],
                                         pair[:, 2 * i, :ksz],
                                         pair[:, 2 * i + 1, :ksz])
                nc.vector.tensor_add(octs[:, h, :ksz], quad[:, 0, :ksz],
                                     quad[:, 1, :ksz])
            hexs = attn2.tile([128, 2, 512], BF16, tag="hex")
            for i in range(2):
                nc.vector.tensor_add(hexs[:, i, :ksz], octs[:, 2 * i, :ksz],
                                     octs[:, 2 * i + 1, :ksz])
            top = attn2.tile([128, 512], BF16, tag="top")
            nc.vector.tensor_add(top[:, :ksz], hexs[:, 0, :ksz],
                                 hexs[:, 1, :ksz])
            x0p = psum1.tile([128, 512], F32, tag="ps_acc")
            ssum = ssum_t[0:1, :]
            for m in range(32):
                eSm = expS_h[m // 8][:, m % 8, :ksz]
                nc.tensor.matmul(x0p[:, :ksz], x1t_s[:, m, :], eSm,
                                 start=(m == 0), stop=(m == 31))
            nc.tensor.matmul(ssum[:, :ksz], ones_s, top[:, :ksz],
                             start=True, stop=True)
            sinv = small.tile([1, 512], F32, tag="sinv")
            nc.vector.reciprocal(sinv[:, :ksz], ssum[:, :ksz])
            # two conv0 in0-half partial groups per k-tile fill the PE slack
            emit_in0_partial(2 * kt)
            emit_in0_partial(2 * kt + 1)
            if kt < 4:
                # broadcast 1/colsum to all partitions via a DRAM roundtrip
                # (latency hidden mid-attention)
                sinv_d = dram.tile([1, 512], F32, tag="sinv_d")
                nc.sync.dma_start(out=sinv_d[:, :ksz], in_=sinv[:, :ksz])
                sinvb = small.tile([128, 512], F32, tag="sinvb")
                nc.sync.dma_start(
                    out=sinvb[:, :ksz],
                    in_=sinv_d[:, :ksz].partition_broadcast(128)[:, 0, :])
                nc.vector.tensor_mul(x0n_s[:, k0:k0 + ksz], x0p[:, :ksz],
                                     sinvb[:, :ksz])
            else:
                # last tile feeds the serial attention->conv transition:
                # broadcast via a tiny fp32 ones-matmul instead (no DMA
                # latency on the critical path)
                bcast = psum1.tile([128, 512], F32, tag="ps_cv",
                                   name="sinv_bcast")
                nc.tensor.matmul(bcast[:, :ksz], ones_row, sinv[:, :ksz],
                                 start=True, stop=True)
                sinvb = small.tile([128, 512], F32, tag="sinvb")
                nc.vector.tensor_copy(sinvb[:, :ksz], bcast[:, :ksz])
                nc.vector.tensor_mul(x0n_s[:, k0:k0 + ksz], x0p[:, :ksz],
                                     sinvb[:, :ksz])

        # ---- out-projection + bn0 + gelu -> x0' (fp8) into ybuf ----
        for kt, (k0, ksz) in enumerate(K_TILES):
            nr = ksz // W  # rows in this k-tile
            for oc in range(2):
                po = psum.tile([128, 2, 512], F32, tag="ps_S")
                nc.tensor.matmul(po[:, 0, :ksz],
                                 wo_s[:, oc * 128:(oc + 1) * 128],
                                 x0n_s[:, k0:k0 + ksz],
                                 start=True, stop=True)
                nc.scalar.activation(
                    ybuf[:, oc, 1 + kt * 8:1 + kt * 8 + nr, 1:65],
                    po[:, 0, :ksz].rearrange("p (r w) -> p r w", w=W),
                    AF.Gelu, bias=bo_s[:, oc:oc + 1], scale=ao_s[:, oc:oc + 1])

        # ---- conv1 weights (loaded during attention; fp8 hi/lo) ----
        w1all_s = persist.tile([128, 36, C], F8, tag="w1all")
        nc.sync.dma_start(
            out=w1all_s,
            in_=w1all_d.rearrange("s t (a p) o -> p (s t a) o", a=2))
        w1h_s = w1all_s[:, 0:18]
        w1l_s = w1all_s[:, 18:36]

        # ---- conv0: x0'-half naive fp8 DoubleRow on top of the re-injected
        # in0-half partial (identity matmul opens the accumulation) ----
        for ri, (r0, nr) in enumerate(ROW_TILES):
            for oc in range(2):
                pc = psum.tile([128, 2, 512], F32, tag="ps_S")
                pcv = pc[:, 0, :nr * W].rearrange("p (r w) -> p r w", w=W)
                nc.tensor.matmul(pc[:, 0, :nr * W], id_s,
                                 inpart[ri * 2 + oc][:, :nr * W],
                                 start=True, stop=False)
                for t9 in range(9):
                    dh, dw = divmod(t9, 3)
                    nc.tensor.matmul(
                        pcv,
                        w0x_s[:, t9 * 2:t9 * 2 + 2, oc * 128:(oc + 1) * 128],
                        ybuf[:, :, r0 + dh - 1:r0 + dh - 1 + nr, dw:dw + W],
                        start=False, stop=(t9 == 8),
                        perf_mode=DR)
                nc.scalar.activation(
                    c0f[:, oc, r0 - 1:r0 - 1 + nr, :], pcv,
                    AF.Gelu, bias=b0_s[:, oc:oc + 1], scale=a0_s[:, oc:oc + 1])
                # hi/lo split of c0 for conv1's 3-term product (DVE)
                nc.vector.tensor_copy(c0h[:, oc, r0:r0 + nr, 1:65],
                                      c0f[:, oc, r0 - 1:r0 - 1 + nr, :])
                nc.vector.tensor_sub(c0l[:, oc, r0:r0 + nr, 1:65],
                                     c0f[:, oc, r0 - 1:r0 - 1 + nr, :],
                                     c0h[:, oc, r0:r0 + nr, 1:65])

        # ---- conv1: 256 -> 256, 3-term DoubleRow fp8, bn + gelu,
        #      + x0' residual, row-max; per-row-tile output DMA so only the
        #      small last tile sits on the kernel tail ----
        for r0, nr in ROW_TILES:
            for oc in range(2):
                pc = psum.tile([128, 2, 512], F32, tag="ps_S")
                pcv = pc[:, 0, :nr * W].rearrange("p (r w) -> p r w", w=W)
                terms1 = [(w1h_s, c0h), (w1h_s, c0l), (w1l_s, c0h)]
                i_mm, n_mm = 0, 9 * len(terms1)
                for w_s, x_s in terms1:
                    for t9 in range(9):
                        dh, dw = divmod(t9, 3)
                        nc.tensor.matmul(
                            pcv,
                            w_s[:, t9 * 2:t9 * 2 + 2, oc * 128:(oc + 1) * 128],
                            x_s[:, :, r0 + dh - 1:r0 + dh - 1 + nr, dw:dw + W],
                            start=(i_mm == 0), stop=(i_mm == n_mm - 1),
                            perf_mode=DR)
                        i_mm += 1
                tmp = small.tile([128, 512], F32, tag="scratch")
                nc.scalar.activation(tmp[:, :nr * W], pc[:, 0, :nr * W], AF.Gelu,
                                     bias=b1_s[:, oc:oc + 1],
                                     scale=a1_s[:, oc:oc + 1])
                res = small.tile([128, 512], F32, tag="scratch")
                nc.vector.tensor_add(
                    res[:, :nr * W].rearrange("p (r w) -> p r w", w=W),
                    tmp[:, :nr * W].rearrange("p (r w) -> p r w", w=W),
                    ybuf[:, oc, r0:r0 + nr, 1:65])
                outr = small.tile([128, 8], F32, tag="outr")
                nc.vector.reduce_max(
                    outr[:, :nr],
                    res[:, :nr * W].rearrange("p (r w) -> p r w", w=W),
                    axis=AX.X)
                # alternate HWDGE queues so the two last-tile DMAs overlap
                eng = nc.sync if oc == 0 else nc.scalar
                eng.dma_start(
                    out=out[oc * 128:(oc + 1) * 128, r0 - 1:r0 - 1 + nr],
                    in_=outr[:, :nr])

    nc.compile()
    return nc


def _prep_maps(inputs):
    """Host-side input prep: slicing, transposes, BN folding, fp8 splits."""
    f = np.float32
    in0 = np.ascontiguousarray(np.asarray(inputs["inputs_0"], f).reshape(B, C, N))
    in1 = np.ascontiguousarray(np.asarray(inputs["inputs_1"], f).reshape(B, C, N))
    g = {k: np.asarray(v, f) for k, v in inputs.items()}

    def fold(gm, bt, m, v, conv_b):
        a = (gm / np.sqrt(v + EPS)).astype(f)
        return a, (bt - m * a + a * conv_b).astype(f)

    a_bn, b_bn = fold(g["bn0_g"], g["bn0_b"], g["bn0_m"], g["bn0_v"],
                      g["b_o"] + g["w_o"] @ g["b_q"])
    a0, b0 = fold(g["cb_bn0_g"], g["cb_bn0_b"], g["cb_bn0_m"], g["cb_bn0_v"],
                  g["cb_b0"])
    a1, b1 = fold(g["cb_bn1_g"], g["cb_bn1_b"], g["cb_bn1_m"], g["cb_bn1_v"],
                  g["cb_b1"])

    def wsplit(w):
        wh = w.astype(F8NP)
        wl = (w - wh.astype(f)).astype(F8NP)
        return wh, wl

    # conv weights as (tap, ci, o); x0-half naive fp8, in0-half + w1 hi/lo
    w0t = np.ascontiguousarray(
        g["cb_w0"].transpose(2, 3, 1, 0).reshape(9, 2 * C, C))
    w1t = np.ascontiguousarray(
        g["cb_w1"].transpose(2, 3, 1, 0).reshape(9, C, C))
    w0inh, w0inl = wsplit(w0t[:, C:, :])
    w1h, w1l = wsplit(w1t)

    shared = {
        "wq_t": np.ascontiguousarray(g["w_q"].T).astype(BF16NP),
        "wk_t": np.ascontiguousarray(g["w_k"].T).astype(BF16NP),
        "wv_t": np.ascontiguousarray(g["w_v"].T).astype(BF16NP),
        "wo_t": np.ascontiguousarray(g["w_o"].T).astype(BF16NP),
        "bv": np.ascontiguousarray(g["b_v"].reshape(Ch, 1)),
        "bias6": np.ascontiguousarray(
            np.stack([a_bn, b_bn, a0, b0, a1, b1]).reshape(6, C, 1)),
        "w0all": np.ascontiguousarray(
            np.stack([w0t[:, :C, :].astype(F8NP), w0inh, w0inl])),
        "w1all": np.ascontiguousarray(np.stack([w1h, w1l])),
        "idm": np.eye(128, dtype=BF16NP),
    }
    maps = []
    for b in range(B):
        in0b16 = in0[b].astype(BF16NP)
        for half in range(2):
            w0r = 0 if half == 0 else 30
            sl = slice(w0r * W, (w0r + ROWS) * W)
            in0w_f32 = in0[b][:, sl].reshape(C, ROWS, W)
            in0h = np.zeros((C, ROWS, 66), F8NP)
            in0l = np.zeros((C, ROWS, 66), F8NP)
            in0h[:, :, 1:65] = in0w_f32.astype(F8NP)
            in0l[:, :, 1:65] = (
                in0w_f32 - in0h[:, :, 1:65].astype(f)).astype(F8NP)
            maps.append({
                "in0b": in0b16,
                "in0h": in0h.reshape(C, ROWS * 66),
                "in0l": in0l.reshape(C, ROWS * 66),
                "in1b": np.ascontiguousarray(in1[b][:, sl]).astype(BF16NP),
                **shared,
            })
    return maps


def kernel(**inputs):
    if "nc" not in _CACHED:
        _CACHED["nc"] = build_program()
    nc = _CACHED["nc"]
    maps = _prep_maps(inputs)
    res = run_bass_kernel_spmd(nc, maps, core_ids=list(range(8)))
    out = np.zeros((B, C), np.float32)
    for b in range(B):
        top = res.results[2 * b]["out"][:, 0:32].max(axis=1)
        bot = res.results[2 * b + 1]["out"][:, 2:34].max(axis=1)
        out[b] = np.maximum(out[b], np.maximum(top, bot))
    return out


# revision 31
# speedup vs baseline: 1.0133x; 1.0133x over previous
"""Trainium2 Bass kernel for nn_CFAConv (cross-feature attention + conv block).

Self-contained: takes full unsharded inputs, shards (batch, image-half) across
8 NeuronCores, runs one SPMD Bass/Tile NEFF, and combines partial results on
the host.

Math (validated against the jax reference in numpy):
  x1 = w_q@in0 + b_q ; x2 = w_k@in0 + b_k ; x3 = w_v@in1 + b_v  (1x1 convs)
  aff = softmax_j(x2^T x3) ; x0 = x1 @ aff
  x0' = gelu(bn0(w_o@x0 + b_o))
  y = gelu(bn(conv3x3(concat(x0', in0)))) ; y = gelu(bn(conv3x3(y)))
  out = max_spatial(y + x0')
On-device simplifications:
  - softmax over j is invariant to per-column shifts => b_k drops entirely
  - x2^T(x3 + b_v) = x2^T x3 + (x2^T b_v) 1^T    => fold b_v into x3
  - (x1 + b_q 1^T) @ aff = x1@aff + b_q 1^T (aff columns sum to 1)
    => fold w_o@b_q into the out-projection bias (host-side)
  - eval-mode BN folds to per-channel scale/bias, fused into the gelu ACT op
  - softmax normalization deferred past the x1@exp(S) matmul (divide x0 by
    column sums); sums via a 5-level bf16 DVE pre-sum tree + one ones-matmul
  - no max-subtraction in softmax: |S| <= ~60 here; exp fits fp32 (max ~e88)
Precision: bf16 operands with fp32 PSUM accumulation for the attention path;
the two 3x3 convs run in fp8e4m3 with DoubleRow perf mode (2 contraction
tiles per pass at 0.5 cycles/row):
  - conv0 x0'-half: weights + acts naive fp8 (x0' is small vs in0 => cheap)
  - conv0 in0-half: weights hi+lo fp8 split, in0 hi+lo fp8 split (host-side),
    3-term product (Wh Xh + Wh Xl + Wl Xh)
  - conv1: weights hi+lo (host), c0 hi+lo split on DVE, 3-term
  (numpy bit-model: 1.3e-2 final rel err vs the 2e-2 budget)
Sharding: 8 cores = (4 batches) x (top/bottom image half). Each core computes
a 34-row window (32 owned + halo) so the two 3x3 convs need no communication;
per-row maxes [256, 34] go to the host which slices owned rows and reduces.
"""

from contextlib import ExitStack

import ml_dtypes
import numpy as np

import concourse.bass as bass
import concourse.tile as tile
from concourse import bacc, mybir
from concourse.bass_utils import run_bass_kernel_spmd

B, C, H, W = 4, 256, 64, 64
Ch = C // 2          # 128
N = H * W            # 4096
ROWS = 34            # per-core row window (32 owned + 2 halo)
KW = ROWS * W        # 2176 window positions
EPS = 1e-5

F32 = mybir.dt.float32
BF16 = mybir.dt.bfloat16
F8 = mybir.dt.float8e4
AF = mybir.ActivationFunctionType
AX = mybir.AxisListType
DR = mybir.MatmulPerfMode.DoubleRow
BF16NP = ml_dtypes.bfloat16
F8NP = ml_dtypes.float8_e4m3

# attention k-tiles over the 2176-column window
K_TILES = [(0, 512), (512, 512), (1024, 512), (1536, 512), (2048, 128)]
# conv output row-tiles (local rows 1..34 of the 36-row padded buffer)
ROW_TILES = [(1, 8), (9, 8), (17, 8), (25, 8), (33, 2)]

_CACHED = {}


def build_program():
    nc = bacc.Bacc("TRN2", target_bir_lowering=False, debug=False)

    def din(name, shape, dt=F32):
        return nc.dram_tensor(name, shape, dt, kind="ExternalInput").ap()

    in0b_d = din("in0b", [C, N], BF16)
    in1b_d = din("in1b", [C, KW], BF16)
    # in0 conv window, fp8 hi/lo, pre-padded to 66 cols (zero side columns)
    in0h_d = din("in0h", [C, ROWS * 66], F8)
    in0l_d = din("in0l", [C, ROWS * 66], F8)
    wq_t = din("wq_t", [C, Ch], BF16)     # (c, i)
    wk_t = din("wk_t", [C, Ch], BF16)
    wv_t = din("wv_t", [C, Ch], BF16)
    wo_t = din("wo_t", [Ch, C], BF16)     # (i, o)
    bv = din("bv", [Ch, 1])
    bias6_d = din("bias6", [6, C, 1])     # ao, bo, a0, b0, a1, b1
    # conv0 weights: [x0-half naive, in0-half hi, in0-half lo] (tap, ci, o)
    w0all_d = din("w0all", [3, 9, C, C], F8)
    w1all_d = din("w1all", [2, 9, C, C], F8)  # [hi, lo]
    idm_d = din("idm", [128, 128], BF16)
    out = nc.dram_tensor("out", [C, ROWS], F32, kind="ExternalOutput").ap()

    with tile.TileContext(nc) as tc, ExitStack() as ctx:
        persist = ctx.enter_context(tc.tile_pool(name="persist", bufs=1))
        psum = ctx.enter_context(tc.tile_pool(name="psum", bufs=2, space="PSUM"))
        psum1 = ctx.enter_context(tc.tile_pool(name="psum1", bufs=2, space="PSUM"))
        small = ctx.enter_context(tc.tile_pool(name="small", bufs=3))

        # ---- inputs: bf16 quarters of in0 (one DMA each: per-slice deps
        # because Tile dependencies are whole-tile). Weights + a small first
        # slice of in0 go first so the first matmul starts ASAP. ----
        wk_s = persist.tile([128, 2, Ch], BF16, tag="wk")
        nc.sync.dma_start(out=wk_s, in_=wk_t.rearrange("(a p) n -> p a n", a=2))
        wq_s = persist.tile([128, 2, Ch], BF16, tag="wq")
        nc.sync.dma_start(out=wq_s, in_=wq_t.rearrange("(a p) n -> p a n", a=2))
        in0q0a = persist.tile([128, 2, 128], BF16, tag="in0q0a")
        nc.sync.dma_start(
            out=in0q0a,
            in_=in0b_d.rearrange("(a p) n -> p a n", a=2)[:, :, 0:128])
        in0q0b = persist.tile([128, 2, 384], BF16, tag="in0q0b")
        nc.sync.dma_start(
            out=in0q0b,
            in_=in0b_d.rearrange("(a p) n -> p a n", a=2)[:, :, 128:512])
        in0q = [None] + [persist.tile([128, 2, 512], BF16, tag=f"in0q{q}",
                                      name=f"in0q{q}") for q in range(1, 8)]
        in1q = [persist.tile([128, 2, 1152], BF16, tag=f"in1q{q}",
                             name=f"in1q{q}") for q in range(2)]
        # odd/even split of the in0 quarters across the two HWDGE queues (ACT
        # is idle until the exps) so arrivals keep pace with the projections
        for q in (1, 2, 4, 6):
            nc.scalar.dma_start(
                out=in0q[q],
                in_=in0b_d.rearrange("(a p) n -> p a n", a=2)[
                    :, :, q * 512:(q + 1) * 512])
        nc.scalar.dma_start(
            out=in1q[0][:, :, :1024],
            in_=in1b_d.rearrange("(a p) n -> p a n", a=2)[:, :, 0:1024])
        nc.scalar.dma_start(
            out=in1q[1],
            in_=in1b_d.rearrange("(a p) n -> p a n", a=2)[:, :, 1024:KW])
        for q in (3, 5, 7):
            nc.sync.dma_start(
                out=in0q[q],
                in_=in0b_d.rearrange("(a p) n -> p a n", a=2)[
                    :, :, q * 512:(q + 1) * 512])
        wv_s = persist.tile([128, 2, Ch], BF16, tag="wv")
        nc.sync.dma_start(out=wv_s, in_=wv_t.rearrange("(a p) n -> p a n", a=2))
        bv_s = persist.tile([128, 1], F32, tag="bv")
        nc.sync.dma_start(out=bv_s, in_=bv)
        wo_s = persist.tile([128, C], BF16, tag="wo")
        nc.sync.dma_start(out=wo_s, in_=wo_t)
        bias_s = persist.tile([128, 12], F32, tag="bias6")
        nc.sync.dma_start(out=bias_s,
                          in_=bias6_d.rearrange("t (a p) o -> p (t a o)", a=2))
        ao_s, bo_s = bias_s[:, 0:2], bias_s[:, 2:4]
        a0_s, b0_s = bias_s[:, 4:6], bias_s[:, 6:8]
        a1_s, b1_s = bias_s[:, 8:10], bias_s[:, 10:12]
        ones_s = persist.tile([128, 1], BF16, tag="ones")
        nc.vector.memset(ones_s, 1.0)
        ones_row = persist.tile([1, 128], F32, tag="ones_row")
        nc.vector.memset(ones_row, 1.0)
        id_s = persist.tile([128, 128], BF16, tag="idm")
        nc.sync.dma_start(out=id_s, in_=idm_d)

        # ---- projections: x2 [ch, N], x1T [j, i], x3 [ch, KW] (all bf16) --
        x2_s = persist.tile([128, N], BF16, tag="x2")
        x1t_s = persist.tile([128, 32, Ch], BF16, tag="x1t")
        x3_s = persist.tile([128, KW], BF16, tag="x3")

        def jc_slices(jc, lo, hi):
            """moving-operand slices [lo:hi) of in0 quarter jc (jc 0 is split
            into a 128-col head so the first matmul follows a small DMA)."""
            if jc > 0:
                return [(in0q[jc], lo, hi, lo)]
            out = []
            if lo < 128:
                out.append((in0q0a, lo, min(hi, 128), lo))
            if hi > 128:
                out.append((in0q0b, max(lo, 128) - 128, hi - 128,
                            max(lo, 128)))
            return out

        for jc in range(8):
            # x2 chunk: one accumulation group in one PSUM bank
            ps2 = psum.tile([128, 2, 512], F32, tag="ps_S")
            mms = [(t, a, b, o, cc) for cc in range(2)
                   for (t, a, b, o) in jc_slices(jc, 0, 512)]
            for i, (t, a, b, o, cc) in enumerate(mms):
                nc.tensor.matmul(ps2[:, 0, o:o + b - a], wk_s[:, cc, :],
                                 t[:, cc, a:b],
                                 start=(i == 0), stop=(i == len(mms) - 1))
            nc.vector.tensor_copy(x2_s[:, jc * 512:(jc + 1) * 512], ps2[:, 0, :])
            # x1t: all 4 j-subchunks in ONE group/bank, evicted in one ACT copy
            ps1 = psum1.tile([128, 512], F32, tag="ps_acc")
            mms = [(js, t, a, b, cc) for js in range(4) for cc in range(2)
                   for (t, a, b, _) in jc_slices(jc, js * 128, (js + 1) * 128)]
            for i, (js, t, a, b, cc) in enumerate(mms):
                nc.tensor.matmul(
                    ps1[:, js * 128:js * 128 + Ch],
                    t[:, cc, a:b], wq_s[:, cc, :],
                    start=(i == 0), stop=(i == len(mms) - 1))
            if jc < 3:  # ACT is dispatching input DMAs early on; use DVE
                nc.vector.tensor_copy(
                    x1t_s[:, jc * 4:(jc + 1) * 4, :],
                    ps1.rearrange("p (a c) -> p a c", c=Ch))
            else:
                nc.scalar.activation(
                    x1t_s[:, jc * 4:(jc + 1) * 4, :],
                    ps1.rearrange("p (a c) -> p a c", c=Ch), AF.Copy)

        for k0, ksz in K_TILES:
            iq, off = (0, k0) if k0 < 1024 else (1, k0 - 1024)
            ps3 = psum.tile([128, 2, 512], F32, tag="ps_S")
            for cc in range(2):
                nc.tensor.matmul(ps3[:, 0, :ksz], wv_s[:, cc, :],
                                 in1q[iq][:, cc, off:off + ksz],
                                 start=(cc == 0), stop=(cc == 1))
            # x3 = psum + b_v : folds the v-bias into the affinity logits
            nc.vector.tensor_scalar_add(x3_s[:, k0:k0 + ksz], ps3[:, 0, :ksz],
                                        bv_s)

        # ---- conv buffers (fp8, padded 36x66 with zero ring) ----
        convbuf = ctx.enter_context(tc.tile_pool(name="convbuf", bufs=1))
        ybuf = convbuf.tile([128, 2, 36, 66], F8, tag="ybuf")   # x0' chunks
        in0h_s = convbuf.tile([128, 2, 36, 66], F8, tag="in0h")
        in0l_s = convbuf.tile([128, 2, 36, 66], F8, tag="in0l")
        c0h = convbuf.tile([128, 2, 36, 66], F8, tag="c0h")
        c0l = convbuf.tile([128, 2, 36, 66], F8, tag="c0l")
        c0f = convbuf.tile([128, 2, ROWS, W], BF16, tag="c0f")
        for tl in (ybuf, in0h_s, in0l_s, c0h, c0l):
            # zero the pad ring (write-only memset; reading uninitialized
            # SBUF can produce NaNs)
            nc.vector.memset(tl[:, :, 0, :], 0.0)
            nc.vector.memset(tl[:, :, 35, :], 0.0)
            if tl is in0h_s or tl is in0l_s:
                continue  # side columns arrive zero-padded via the DMA
            nc.vector.memset(tl[:, :, 1:35, 0:1], 0.0)
            nc.vector.memset(tl[:, :, 1:35, 65:66], 0.0)
        # in0 conv window ships as fp8 hi/lo straight into the padded tiles
        # (host pre-pads the 66-col side ring so the DMA stays 3-dim)
        nc.sync.dma_start(
            out=in0h_s[:, :, 1:35, :],
            in_=in0h_d.rearrange("(a p) n -> p a n", a=2))
        nc.sync.dma_start(
            out=in0l_s[:, :, 1:35, :],
            in_=in0l_d.rearrange("(a p) n -> p a n", a=2))

        # ---- conv0 weights (early: the in0-half partial sums run inside the
        # attention phase to fill PE slack while ACT grinds the exps) ----
        w0all_s = persist.tile([128, 54, C], F8, tag="w0all")
        nc.sync.dma_start(
            out=w0all_s,
            in_=w0all_d.rearrange("s t (a p) o -> p (s t a) o", a=2))
        w0x_s = w0all_s[:, 0:18]
        w0inh_s = w0all_s[:, 18:36]
        w0inl_s = w0all_s[:, 36:54]
        inpart = [persist.tile([128, 512], BF16, tag=f"inpart{g}",
                               name=f"inpart{g}") for g in range(10)]
        terms_in0 = [(w0inh_s, in0h_s), (w0inh_s, in0l_s), (w0inl_s, in0h_s)]

        def emit_in0_partial(g):
            """27 DoubleRow passes of conv0's in0-half for group g=(rt,oc),
            evicted to SBUF bf16 for later re-injection."""
            (r0, nr), oc = ROW_TILES[g // 2], g % 2
            ps = psum1.tile([128, 512], F32, tag="ps_cv", name=f"cv{g}")
            pcv = ps[:, :nr * W].rearrange("p (r w) -> p r w", w=W)
            i_mm, n_mm = 0, 27
            for w_s, x_s in terms_in0:
                for t9 in range(9):
                    dh, dw = divmod(t9, 3)
                    nc.tensor.matmul(
                        pcv,
                        w_s[:, t9 * 2:t9 * 2 + 2, oc * 128:(oc + 1) * 128],
                        x_s[:, :, r0 + dh - 1:r0 + dh - 1 + nr, dw:dw + W],
                        start=(i_mm == 0), stop=(i_mm == n_mm - 1),
                        perf_mode=DR)
                    i_mm += 1
            nc.vector.tensor_copy(inpart[g][:, :nr * W], ps[:, :nr * W])

        # ---- attention: S = x2^T x3, exp, x0 = x1 @ exp, sums, normalize ----
        attn = ctx.enter_context(tc.tile_pool(name="attn", bufs=4))
        attn2 = ctx.enter_context(tc.tile_pool(name="attn2", bufs=2))
        dram = ctx.enter_context(tc.tile_pool(name="dram", bufs=5, space="DRAM"))
        x0n_s = persist.tile([128, KW], BF16, tag="x0n")
        for kt, (k0, ksz) in enumerate(K_TILES):
            # four quarter-tiles under one bufs=4 tag: stage-2 consumes a
            # quarter while later quarters' exps still run, and the next
            # k-tile's exps begin as soon as a quarter is drained
            expS_h = [attn.tile([128, 8, 512], BF16, tag="expS",
                                name=f"expS{k0}_{h}") for h in range(4)]
            # ssum shares the ps_cv ring with the conv0 in0-half partials:
            # both have fast consumers so the rotation never stalls the PE
            ssum_t = psum1.tile([128, 512], F32, tag="ps_cv",
                                name=f"ssum{k0}")
            for mh in range(16):  # chunk pairs
                sp = psum.tile([128, 2, 512], F32, tag="ps_S")
                for i in range(2):
                    m = 2 * mh + i
                    nc.tensor.matmul(
                        sp[:, i, :ksz],
                        x2_s[:, m * 128:(m + 1) * 128],
                        x3_s[:, k0:k0 + ksz],
                        start=True, stop=True)
                eh = expS_h[mh // 4]
                nc.scalar.activation(
                    eh[:, (2 * mh) % 8:(2 * mh) % 8 + 2, :ksz],
                    sp[:, :, :ksz], AF.Exp)
            # 5-level bf16 pre-sum tree on DVE collapses the softmax
            # column-sum to ONE ones-matmul pass (sum error ~0.3%, only
            # scales the normalization)
            octs = attn2.tile([128, 4, 512], BF16, tag="oct")
            for h in range(4):
                pair = attn.tile([128, 4, 512], BF16, tag="pair",
                                 name=f"pair{k0}_{h}")
                for i in range(4):
                    nc.vector.tensor_add(pair[:, i, :ksz],
                                         expS_h[h][:, 2 * i, :ksz],
                                         expS_h[h][:, 2 * i + 1, :ksz])
                quad = attn.tile([128, 2, 512], BF16, tag="quad",
                                 name=f"quad{k0}_{h}")
                for i in range(2):
                    nc.vector.tensor_add(quad[:, i, :ksz],
                                         pair[:, 2 * i, :ksz],
                                         pair[:, 2 * i + 1, :ksz])
                nc.vector.tensor_add(octs[:, h, :ksz], quad[:, 0, :ksz],
                                     quad[:, 1, :ksz])
            hexs = attn2.tile([128, 2, 512], BF16, tag="hex")
            for i in range(2):
                nc.vector.tensor_add(hexs[:, i, :ksz], octs[:, 2 * i, :ksz],
                                     octs[:, 2 * i + 1, :ksz])
            top = attn2.tile([128, 512], BF16, tag="top")
            nc.vector.tensor_add(top[:, :ksz], hexs[:, 0, :ksz],
                                 hexs[:, 1, :ksz])
            x0p = psum1.tile([128, 512], F32, tag="ps_acc")
            ssum = ssum_t[0:1, :]
            for m in range(32):
                eSm = expS_h[m // 8][:, m % 8, :ksz]
                nc.tensor.matmul(x0p[:, :ksz], x1t_s[:, m, :], eSm,
                                 start=(m == 0), stop=(m == 31))
            nc.tensor.matmul(ssum[:, :ksz], ones_s, top[:, :ksz],
                             start=True, stop=True)
            sinv = small.tile([1, 512], F32, tag="sinv")
            nc.vector.reciprocal(sinv[:, :ksz], ssum[:, :ksz])
            # two conv0 in0-half partial groups per k-tile fill the PE slack
            emit_in0_partial(2 * kt)
            emit_in0_partial(2 * kt + 1)
            if kt < 4:
                # broadcast 1/colsum to all partitions via a DRAM roundtrip
                # (latency hidden mid-attention)
                sinv_d = dram.tile([1, 512], F32, tag="sinv_d")
                nc.sync.dma_start(out=sinv_d[:, :ksz], in_=sinv[:, :ksz])
                sinvb = small.tile([128, 512], F32, tag="sinvb")
                nc.sync.dma_start(
                    out=sinvb[:, :ksz],
                    in_=sinv_d[:, :ksz].partition_broadcast(128)[:, 0, :])
                nc.vector.tensor_mul(x0n_s[:, k0:k0 + ksz], x0p[:, :ksz],
                                     sinvb[:, :ksz])
            else:
                # last tile feeds the serial attention->conv transition:
                # broadcast via a tiny fp32 ones-matmul instead (no DMA
                # latency on the critical path)
                bcast = psum1.tile([128, 512], F32, tag="ps_cv",
                                   name="sinv_bcast")
                nc.tensor.matmul(bcast[:, :ksz], ones_row, sinv[:, :ksz],
                                 start=True, stop=True)
                sinvb = small.tile([128, 512], F32, tag="sinvb")
                nc.vector.tensor_copy(sinvb[:, :ksz], bcast[:, :ksz])
                nc.vector.tensor_mul(x0n_s[:, k0:k0 + ksz], x0p[:, :ksz],
                                     sinvb[:, :ksz])

        # ---- out-projection + bn0 + gelu -> x0' (fp8) into ybuf ----
        for kt, (k0, ksz) in enumerate(K_TILES):
            nr = ksz // W  # rows in this k-tile
            for oc in range(2):
                po = psum.tile([128, 2, 512], F32, tag="ps_S")
                nc.tensor.matmul(po[:, 0, :ksz],
                                 wo_s[:, oc * 128:(oc + 1) * 128],
                                 x0n_s[:, k0:k0 + ksz],
                                 start=True, stop=True)
                nc.scalar.activation(
                    ybuf[:, oc, 1 + kt * 8:1 + kt * 8 + nr, 1:65],
                    po[:, 0, :ksz].rearrange("p (r w) -> p r w", w=W),
                    AF.Gelu, bias=bo_s[:, oc:oc + 1], scale=ao_s[:, oc:oc + 1])

        # ---- conv1 weights (loaded during attention; fp8 hi/lo) ----
        w1all_s = persist.tile([128, 36, C], F8, tag="w1all")
        nc.sync.dma_start(
            out=w1all_s,
            in_=w1all_d.rearrange("s t (a p) o -> p (s t a) o", a=2))
        w1h_s = w1all_s[:, 0:18]
        w1l_s = w1all_s[:, 18:36]

        # ---- conv0: x0'-half naive fp8 DoubleRow on top of the re-injected
        # in0-half partial (identity matmul opens the accumulation) ----
        for ri, (r0, nr) in enumerate(ROW_TILES):
            for oc in range(2):
                pc = psum.tile([128, 2, 512], F32, tag="ps_S")
                pcv = pc[:, 0, :nr * W].rearrange("p (r w) -> p r w", w=W)
                nc.tensor.matmul(pc[:, 0, :nr * W], id_s,
                                 inpart[ri * 2 + oc][:, :nr * W],
                                 start=True, stop=False)
                for t9 in range(9):
                    dh, dw = divmod(t9, 3)
                    nc.tensor.matmul(
                        pcv,
                        w0x_s[:, t9 * 2:t9 * 2 + 2, oc * 128:(oc + 1) * 128],
                        ybuf[:, :, r0 + dh - 1:r0 + dh - 1 + nr, dw:dw + W],
                        start=False, stop=(t9 == 8),
                        perf_mode=DR)
                nc.scalar.activation(
                    c0f[:, oc, r0 - 1:r0 - 1 + nr, :], pcv,
                    AF.Gelu, bias=b0_s[:, oc:oc + 1], scale=a0_s[:, oc:oc + 1])
                # hi/lo split of c0 for conv1's 3-term product (DVE)
                nc.vector.tensor_copy(c0h[:, oc, r0:r0 + nr, 1:65],
                                      c0f[:, oc, r0 - 1:r0 - 1 + nr, :])
                nc.vector.tensor_sub(c0l[:, oc, r0:r0 + nr, 1:65],
                                     c0f[:, oc, r0 - 1:r0 - 1 + nr, :],
                                     c0h[:, oc, r0:r0 + nr, 1:65])

        # ---- conv1: 256 -> 256, 3-term DoubleRow fp8, bn + gelu,
        #      + x0' residual, row-max; per-row-tile output DMA so only the
        #      small last tile sits on the kernel tail ----
        for r0, nr in ROW_TILES:
            for oc in range(2):
                pc = psum.tile([128, 2, 512], F32, tag="ps_S")
                pcv = pc[:, 0, :nr * W].rearrange("p (r w) -> p r w", w=W)
                terms1 = [(w1h_s, c0h), (w1h_s, c0l), (w1l_s, c0h)]
                i_mm, n_mm = 0, 9 * len(terms1)
                for w_s, x_s in terms1:
                    for t9 in range(9):
                        dh, dw = divmod(t9, 3)
                        nc.tensor.matmul(
                            pcv,
                            w_s[:, t9 * 2:t9 * 2 + 2, oc * 128:(oc + 1) * 128],
                            x_s[:, :, r0 + dh - 1:r0 + dh - 1 + nr, dw:dw + W],
                            start=(i_mm == 0), stop=(i_mm == n_mm - 1),
                            perf_mode=DR)
                        i_mm += 1
                tmp = small.tile([128, 512], F32, tag="scratch")
                nc.scalar.activation(tmp[:, :nr * W], pc[:, 0, :nr * W], AF.Gelu,
                                     bias=b1_s[:, oc:oc + 1],
                                     scale=a1_s[:, oc:oc + 1])
                res = small.tile([128, 512], F32, tag="scratch")
                nc.vector.tensor_add(
                    res[:, :nr * W].rearrange("p (r w) -> p r w", w=W),
                    tmp[:, :nr * W].rearrange("p (r w) -> p r w", w=W),
                    ybuf[:, oc, r0:r0 + nr, 1:65])
                outr = small.tile([128, 8], F32, tag="outr")
                nc.vector.reduce_max(
                    outr[:, :nr],
                    res[:, :nr * W].rearrange("p (r w) -> p r w", w=W),
                    axis=AX.X)
                # alternate HWDGE queues so the two last-tile DMAs overlap
                eng = nc.sync if oc == 0 else nc.scalar
                eng.dma_start(
                    out=out[oc * 128:(oc + 1) * 128, r0 - 1:r0 - 1 + nr],
                    in_=outr[:, :nr])

    nc.compile()
    return nc


def _prep_maps(inputs):
    """Host-side input prep: slicing, transposes, BN folding, fp8 splits."""
    f = np.float32
    in0 = np.ascontiguousarray(np.asarray(inputs["inputs_0"], f).reshape(B, C, N))
    in1 = np.ascontiguousarray(np.asarray(inputs["inputs_1"], f).reshape(B, C, N))
    g = {k: np.asarray(v, f) for k, v in inputs.items()}

    def fold(gm, bt, m, v, conv_b):
        a = (gm / np.sqrt(v + EPS)).astype(f)
        return a, (bt - m * a + a * conv_b).astype(f)

    a_bn, b_bn = fold(g["bn0_g"], g["bn0_b"], g["bn0_m"], g["bn0_v"],
                      g["b_o"] + g["w_o"] @ g["b_q"])
    a0, b0 = fold(g["cb_bn0_g"], g["cb_bn0_b"], g["cb_bn0_m"], g["cb_bn0_v"],
                  g["cb_b0"])
    a1, b1 = fold(g["cb_bn1_g"], g["cb_bn1_b"], g["cb_bn1_m"], g["cb_bn1_v"],
                  g["cb_b1"])

    def wsplit(w):
        wh = w.astype(F8NP)
        wl = (w - wh.astype(f)).astype(F8NP)
        return wh, wl

    # conv weights as (tap, ci, o); x0-half naive fp8, in0-half + w1 hi/lo
    w0t = np.ascontiguousarray(
        g["cb_w0"].transpose(2, 3, 1, 0).reshape(9, 2 * C, C))
    w1t = np.ascontiguousarray(
        g["cb_w1"].transpose(2, 3, 1, 0).reshape(9, C, C))
    w0inh, w0inl = wsplit(w0t[:, C:, :])
    w1h, w1l = wsplit(w1t)

    shared = {
        "wq_t": np.ascontiguousarray(g["w_q"].T).astype(BF16NP),
        "wk_t": np.ascontiguousarray(g["w_k"].T).astype(BF16NP),
        "wv_t": np.ascontiguousarray(g["w_v"].T).astype(BF16NP),
        "wo_t": np.ascontiguousarray(g["w_o"].T).astype(BF16NP),
        "bv": np.ascontiguousarray(g["b_v"].reshape(Ch, 1)),
        "bias6": np.ascontiguousarray(
            np.stack([a_bn, b_bn, a0, b0, a1, b1]).reshape(6, C, 1)),
        "w0all": np.ascontiguousarray(
            np.stack([w0t[:, :C, :].astype(F8NP), w0inh, w0inl])),
        "w1all": np.ascontiguousarray(np.stack([w1h, w1l])),
        "idm": np.eye(128, dtype=BF16NP),
    }
    maps = []
    for b in range(B):
        in0b16 = in0[b].astype(BF16NP)
        for half in range(2):
            w0r = 0 if half == 0 else 30
            sl = slice(w0r * W, (w0r + ROWS) * W)
            in0w_f32 = in0[b][:, sl].reshape(C, ROWS, W)
            in0h = np.zeros((C, ROWS, 66), F8NP)
            in0l = np.zeros((C, ROWS, 66), F8NP)
            in0h[:, :, 1:65] = in0w_f32.astype(F8NP)
            in0l[:, :, 1:65] = (
                in0w_f32 - in0h[:, :, 1:65].astype(f)).astype(F8NP)
            maps.append({
                "in0b": in0b16,
                "in0h": in0h.reshape(C, ROWS * 66),
                "in0l": in0l.reshape(C, ROWS * 66),
                "in1b": np.ascontiguousarray(in1[b][:, sl]).astype(BF16NP),
                **shared,
            })
    return maps


def kernel(**inputs):
    if "nc" not in _CACHED:
        _CACHED["nc"] = build_program()
    nc = _CACHED["nc"]
    maps = _prep_maps(inputs)
    res = run_bass_kernel_spmd(nc, maps, core_ids=list(range(8)))
    out = np.zeros((B, C), np.float32)
    for b in range(B):
        top = res.results[2 * b]["out"][:, 0:32].max(axis=1)
        bot = res.results[2 * b + 1]["out"][:, 2:34].max(axis=1)
        out[b] = np.maximum(out[b], np.maximum(top, bot))
    return out


# revision 33
# speedup vs baseline: 1.0166x; 1.0033x over previous
"""Trainium2 Bass kernel for nn_CFAConv (cross-feature attention + conv block).

Self-contained: takes full unsharded inputs, shards (batch, image-half) across
8 NeuronCores, runs one SPMD Bass/Tile NEFF, and combines partial results on
the host.

Math (validated against the jax reference in numpy):
  x1 = w_q@in0 + b_q ; x2 = w_k@in0 + b_k ; x3 = w_v@in1 + b_v  (1x1 convs)
  aff = softmax_j(x2^T x3) ; x0 = x1 @ aff
  x0' = gelu(bn0(w_o@x0 + b_o))
  y = gelu(bn(conv3x3(concat(x0', in0)))) ; y = gelu(bn(conv3x3(y)))
  out = max_spatial(y + x0')
On-device simplifications:
  - softmax over j is invariant to per-column shifts => b_k drops entirely
  - x2^T(x3 + b_v) = x2^T x3 + (x2^T b_v) 1^T    => fold b_v into x3
  - (x1 + b_q 1^T) @ aff = x1@aff + b_q 1^T (aff columns sum to 1)
    => fold w_o@b_q into the out-projection bias (host-side)
  - eval-mode BN folds to per-channel scale/bias, fused into the gelu ACT op
  - softmax normalization deferred past the x1@exp(S) matmul (divide x0 by
    column sums); sums via a 5-level bf16 DVE pre-sum tree + one ones-matmul
  - no max-subtraction in softmax: |S| <= ~60 here; exp fits fp32 (max ~e88)
Precision: bf16 operands with fp32 PSUM accumulation for the attention path;
the two 3x3 convs run in fp8e4m3 with DoubleRow perf mode (2 contraction
tiles per pass at 0.5 cycles/row):
  - conv0 x0'-half: weights + acts naive fp8 (x0' is small vs in0 => cheap)
  - conv0 in0-half: weights hi+lo fp8 split, in0 hi+lo fp8 split (host-side),
    3-term product (Wh Xh + Wh Xl + Wl Xh)
  - conv1: weights hi+lo (host), c0 hi+lo split on DVE, 3-term
  (numpy bit-model: 1.3e-2 final rel err vs the 2e-2 budget)
Sharding: 8 cores = (4 batches) x (top/bottom image half). Each core computes
a 34-row window (32 owned + halo) so the two 3x3 convs need no communication;
per-row maxes [256, 34] go to the host which slices owned rows and reduces.
"""

from contextlib import ExitStack

import ml_dtypes
import numpy as np

import concourse.bass as bass
import concourse.tile as tile
from concourse import bacc, mybir
from concourse.bass_utils import run_bass_kernel_spmd

B, C, H, W = 4, 256, 64, 64
Ch = C // 2          # 128
N = H * W            # 4096
ROWS = 34            # per-core row window (32 owned + 2 halo)
KW = ROWS * W        # 2176 window positions
EPS = 1e-5

F32 = mybir.dt.float32
BF16 = mybir.dt.bfloat16
F8 = mybir.dt.float8e4
AF = mybir.ActivationFunctionType
AX = mybir.AxisListType
DR = mybir.MatmulPerfMode.DoubleRow
BF16NP = ml_dtypes.bfloat16
F8NP = ml_dtypes.float8_e4m3

# attention k-tiles over the 2176-column window
K_TILES = [(0, 512), (512, 512), (1024, 512), (1536, 512), (2048, 128)]
# conv output row-tiles (local rows 1..34 of the 36-row padded buffer)
ROW_TILES = [(1, 8), (9, 8), (17, 8), (25, 8), (33, 2)]

_CACHED = {}


def build_program():
    nc = bacc.Bacc("TRN2", target_bir_lowering=False, debug=False)

    def din(name, shape, dt=F32):
        return nc.dram_tensor(name, shape, dt, kind="ExternalInput").ap()

    in0b_d = din("in0b", [C, N], BF16)
    in1b_d = din("in1b", [C, KW], BF16)
    # in0 conv window, fp8 hi/lo, pre-padded to 66 cols (zero side columns)
    in0h_d = din("in0h", [C, ROWS * 66], F8)
    in0l_d = din("in0l", [C, ROWS * 66], F8)
    wq_t = din("wq_t", [C, Ch], BF16)     # (c, i)
    wk_t = din("wk_t", [C, Ch], BF16)
    wv_t = din("wv_t", [C, Ch], BF16)
    wo_t = din("wo_t", [Ch, C], BF16)     # (i, o)
    bv = din("bv", [Ch, 1])
    bias6_d = din("bias6", [6, C, 1])     # ao, bo, a0, b0, a1, b1
    # conv0 weights: [x0-half naive, in0-half hi, in0-half lo] (tap, ci, o)
    w0all_d = din("w0all", [3, 9, C, C], F8)
    w1all_d = din("w1all", [2, 9, C, C], F8)  # [hi, lo]
    idm_d = din("idm", [128, 128], BF16)
    out = nc.dram_tensor("out", [C, ROWS], F32, kind="ExternalOutput").ap()

    with tile.TileContext(nc) as tc, ExitStack() as ctx:
        persist = ctx.enter_context(tc.tile_pool(name="persist", bufs=1))
        psum = ctx.enter_context(tc.tile_pool(name="psum", bufs=2, space="PSUM"))
        psum1 = ctx.enter_context(tc.tile_pool(name="psum1", bufs=2, space="PSUM"))
        small = ctx.enter_context(tc.tile_pool(name="small", bufs=3))

        # ---- inputs: bf16 quarters of in0 (one DMA each: per-slice deps
        # because Tile dependencies are whole-tile). Weights + a small first
        # slice of in0 go first so the first matmul starts ASAP. ----
        # DMA issue costs ~1.26us/queue: strictly alternate the projection
        # inputs across the SP and ACT HWDGE queues in consumption order
        in0ap = in0b_d.rearrange("(a p) n -> p a n", a=2)
        in0q0a = persist.tile([128, 2, 128], BF16, tag="in0q0a")
        nc.scalar.dma_start(out=in0q0a, in_=in0ap[:, :, 0:128])
        wk_s = persist.tile([128, 2, Ch], BF16, tag="wk")
        nc.sync.dma_start(out=wk_s, in_=wk_t.rearrange("(a p) n -> p a n", a=2))
        in0q0b = persist.tile([128, 2, 384], BF16, tag="in0q0b")
        nc.scalar.dma_start(out=in0q0b, in_=in0ap[:, :, 128:512])
        wq_s = persist.tile([128, 2, Ch], BF16, tag="wq")
        nc.sync.dma_start(out=wq_s, in_=wq_t.rearrange("(a p) n -> p a n", a=2))
        in0q = [None] + [persist.tile([128, 2, 512], BF16, tag=f"in0q{q}",
                                      name=f"in0q{q}") for q in range(1, 8)]
        in1q = [persist.tile([128, 2, 1152], BF16, tag=f"in1q{q}",
                             name=f"in1q{q}") for q in range(2)]
        for q in range(1, 8):
            eng = nc.scalar if q % 2 == 0 else nc.sync
            eng.dma_start(out=in0q[q], in_=in0ap[:, :, q * 512:(q + 1) * 512])
        nc.sync.dma_start(
            out=in1q[0][:, :, :1024],
            in_=in1b_d.rearrange("(a p) n -> p a n", a=2)[:, :, 0:1024])
        nc.scalar.dma_start(
            out=in1q[1],
            in_=in1b_d.rearrange("(a p) n -> p a n", a=2)[:, :, 1024:KW])
        wv_s = persist.tile([128, 2, Ch], BF16, tag="wv")
        nc.scalar.dma_start(out=wv_s,
                            in_=wv_t.rearrange("(a p) n -> p a n", a=2))
        bv_s = persist.tile([128, 1], F32, tag="bv")
        nc.scalar.dma_start(out=bv_s, in_=bv)
        wo_s = persist.tile([128, C], BF16, tag="wo")
        nc.sync.dma_start(out=wo_s, in_=wo_t)
        bias_s = persist.tile([128, 12], F32, tag="bias6")
        nc.sync.dma_start(out=bias_s,
                          in_=bias6_d.rearrange("t (a p) o -> p (t a o)", a=2))
        ao_s, bo_s = bias_s[:, 0:2], bias_s[:, 2:4]
        a0_s, b0_s = bias_s[:, 4:6], bias_s[:, 6:8]
        a1_s, b1_s = bias_s[:, 8:10], bias_s[:, 10:12]
        ones_s = persist.tile([128, 1], BF16, tag="ones")
        nc.vector.memset(ones_s, 1.0)
        ones_row = persist.tile([1, 128], F32, tag="ones_row")
        nc.vector.memset(ones_row, 1.0)
        id_s = persist.tile([128, 128], BF16, tag="idm")
        nc.sync.dma_start(out=id_s, in_=idm_d)

        # ---- projections: x2 [ch, N], x1T [j, i], x3 [ch, KW] (all bf16) --
        x2_s = persist.tile([128, N], BF16, tag="x2")
        x1t_s = persist.tile([128, 32, Ch], BF16, tag="x1t")
        x3_s = persist.tile([128, KW], BF16, tag="x3")

        def jc_slices(jc, lo, hi):
            """moving-operand slices [lo:hi) of in0 quarter jc (jc 0 is split
            into a 128-col head so the first matmul follows a small DMA)."""
            if jc > 0:
                return [(in0q[jc], lo, hi, lo)]
            out = []
            if lo < 128:
                out.append((in0q0a, lo, min(hi, 128), lo))
            if hi > 128:
                out.append((in0q0b, max(lo, 128) - 128, hi - 128,
                            max(lo, 128)))
            return out

        for jc in range(8):
            # x2 chunk: one accumulation group in one PSUM bank
            ps2 = psum.tile([128, 2, 512], F32, tag="ps_S")
            mms = [(t, a, b, o, cc) for cc in range(2)
                   for (t, a, b, o) in jc_slices(jc, 0, 512)]
            for i, (t, a, b, o, cc) in enumerate(mms):
                nc.tensor.matmul(ps2[:, 0, o:o + b - a], wk_s[:, cc, :],
                                 t[:, cc, a:b],
                                 start=(i == 0), stop=(i == len(mms) - 1))
            nc.vector.tensor_copy(x2_s[:, jc * 512:(jc + 1) * 512], ps2[:, 0, :])
            # x1t: all 4 j-subchunks in ONE group/bank, evicted in one ACT copy
            ps1 = psum1.tile([128, 512], F32, tag="ps_acc")
            mms = [(js, t, a, b, cc) for js in range(4) for cc in range(2)
                   for (t, a, b, _) in jc_slices(jc, js * 128, (js + 1) * 128)]
            for i, (js, t, a, b, cc) in enumerate(mms):
                nc.tensor.matmul(
                    ps1[:, js * 128:js * 128 + Ch],
                    t[:, cc, a:b], wq_s[:, cc, :],
                    start=(i == 0), stop=(i == len(mms) - 1))
            if jc < 3:  # ACT is dispatching input DMAs early on; use DVE
                nc.vector.tensor_copy(
                    x1t_s[:, jc * 4:(jc + 1) * 4, :],
                    ps1.rearrange("p (a c) -> p a c", c=Ch))
            else:
                nc.scalar.activation(
                    x1t_s[:, jc * 4:(jc + 1) * 4, :],
                    ps1.rearrange("p (a c) -> p a c", c=Ch), AF.Copy)

        for k0, ksz in K_TILES:
            iq, off = (0, k0) if k0 < 1024 else (1, k0 - 1024)
            ps3 = psum.tile([128, 2, 512], F32, tag="ps_S")
            for cc in range(2):
                nc.tensor.matmul(ps3[:, 0, :ksz], wv_s[:, cc, :],
                                 in1q[iq][:, cc, off:off + ksz],
                                 start=(cc == 0), stop=(cc == 1))
            # x3 = psum + b_v : folds the v-bias into the affinity logits
            nc.vector.tensor_scalar_add(x3_s[:, k0:k0 + ksz], ps3[:, 0, :ksz],
                                        bv_s)

        # ---- conv buffers (fp8, padded 36x66 with zero ring) ----
        convbuf = ctx.enter_context(tc.tile_pool(name="convbuf", bufs=1))
        ybuf = convbuf.tile([128, 2, 36, 66], F8, tag="ybuf")   # x0' chunks
        in0h_s = convbuf.tile([128, 2, 36, 66], F8, tag="in0h")
        in0l_s = convbuf.tile([128, 2, 36, 66], F8, tag="in0l")
        c0h = convbuf.tile([128, 2, 36, 66], F8, tag="c0h")
        c0l = convbuf.tile([128, 2, 36, 66], F8, tag="c0l")
        c0f = convbuf.tile([128, 2, ROWS, W], BF16, tag="c0f")
        for tl in (ybuf, in0h_s, in0l_s, c0h, c0l):
            # zero the pad ring on the otherwise-idle Pool engine (write-only
            # memset; reading uninitialized SBUF can produce NaNs)
            nc.gpsimd.memset(tl[:, :, 0, :], 0.0)
            nc.gpsimd.memset(tl[:, :, 35, :], 0.0)
            if tl is in0h_s or tl is in0l_s:
                continue  # side columns arrive zero-padded via the DMA
            nc.gpsimd.memset(tl[:, :, 1:35, 0:1], 0.0)
            nc.gpsimd.memset(tl[:, :, 1:35, 65:66], 0.0)
        # in0 conv window ships as fp8 hi/lo straight into the padded tiles
        # (host pre-pads the 66-col side ring so the DMA stays 3-dim)
        nc.sync.dma_start(
            out=in0h_s[:, :, 1:35, :],
            in_=in0h_d.rearrange("(a p) n -> p a n", a=2))
        nc.sync.dma_start(
            out=in0l_s[:, :, 1:35, :],
            in_=in0l_d.rearrange("(a p) n -> p a n", a=2))

        # ---- conv0 weights (early: the in0-half partial sums run inside the
        # attention phase to fill PE slack while ACT grinds the exps) ----
        w0all_s = persist.tile([128, 54, C], F8, tag="w0all")
        nc.sync.dma_start(
            out=w0all_s,
            in_=w0all_d.rearrange("s t (a p) o -> p (s t a) o", a=2))
        w0x_s = w0all_s[:, 0:18]
        w0inh_s = w0all_s[:, 18:36]
        w0inl_s = w0all_s[:, 36:54]
        inpart = [persist.tile([128, 512], BF16, tag=f"inpart{g}",
                               name=f"inpart{g}") for g in range(10)]
        terms_in0 = [(w0inh_s, in0h_s), (w0inh_s, in0l_s), (w0inl_s, in0h_s)]

        def emit_in0_partial(g):
            """27 DoubleRow passes of conv0's in0-half for group g=(rt,oc),
            evicted to SBUF bf16 for later re-injection."""
            (r0, nr), oc = ROW_TILES[g // 2], g % 2
            ps = psum1.tile([128, 512], F32, tag="ps_cv", name=f"cv{g}")
            pcv = ps[:, :nr * W].rearrange("p (r w) -> p r w", w=W)
            i_mm, n_mm = 0, 27
            for w_s, x_s in terms_in0:
                for t9 in range(9):
                    dh, dw = divmod(t9, 3)
                    nc.tensor.matmul(
                        pcv,
                        w_s[:, t9 * 2:t9 * 2 + 2, oc * 128:(oc + 1) * 128],
                        x_s[:, :, r0 + dh - 1:r0 + dh - 1 + nr, dw:dw + W],
                        start=(i_mm == 0), stop=(i_mm == n_mm - 1),
                        perf_mode=DR)
                    i_mm += 1
            nc.vector.tensor_copy(inpart[g][:, :nr * W], ps[:, :nr * W])

        # ---- attention: S = x2^T x3, exp, x0 = x1 @ exp, sums, normalize ----
        attn = ctx.enter_context(tc.tile_pool(name="attn", bufs=4))
        attn2 = ctx.enter_context(tc.tile_pool(name="attn2", bufs=2))
        dram = ctx.enter_context(tc.tile_pool(name="dram", bufs=5, space="DRAM"))
        x0n_s = persist.tile([128, KW], BF16, tag="x0n")
        for kt, (k0, ksz) in enumerate(K_TILES):
            # four quarter-tiles under one bufs=4 tag: stage-2 consumes a
            # quarter while later quarters' exps still run, and the next
            # k-tile's exps begin as soon as a quarter is drained
            expS_h = [attn.tile([128, 8, 512], BF16, tag="expS",
                                name=f"expS{k0}_{h}") for h in range(4)]
            # ssum shares the ps_cv ring with the conv0 in0-half partials:
            # both have fast consumers so the rotation never stalls the PE
            ssum_t = psum1.tile([128, 512], F32, tag="ps_cv",
                                name=f"ssum{k0}")
            for mh in range(16):  # chunk pairs
                sp = psum.tile([128, 2, 512], F32, tag="ps_S")
                for i in range(2):
                    m = 2 * mh + i
                    nc.tensor.matmul(
                        sp[:, i, :ksz],
                        x2_s[:, m * 128:(m + 1) * 128],
                        x3_s[:, k0:k0 + ksz],
                        start=True, stop=True)
                eh = expS_h[mh // 4]
                nc.scalar.activation(
                    eh[:, (2 * mh) % 8:(2 * mh) % 8 + 2, :ksz],
                    sp[:, :, :ksz], AF.Exp)
            # 5-level bf16 pre-sum tree on DVE collapses the softmax
            # column-sum to ONE ones-matmul pass (sum error ~0.3%, only
            # scales the normalization)
            octs = attn2.tile([128, 4, 512], BF16, tag="oct")
            for h in range(4):
                pair = attn.tile([128, 4, 512], BF16, tag="pair",
                                 name=f"pair{k0}_{h}")
                for i in range(4):
                    nc.vector.tensor_add(pair[:, i, :ksz],
                                         expS_h[h][:, 2 * i, :ksz],
                                         expS_h[h][:, 2 * i + 1, :ksz])
                quad = attn.tile([128, 2, 512], BF16, tag="quad",
                                 name=f"quad{k0}_{h}")
                for i in range(2):
                    nc.vector.tensor_add(quad[:, i, :ksz],
                                         pair[:, 2 * i, :ksz],
                                         pair[:, 2 * i + 1, :ksz])
                nc.vector.tensor_add(octs[:, h, :ksz], quad[:, 0, :ksz],
                                     quad[:, 1, :ksz])
            hexs = attn2.tile([128, 2, 512], BF16, tag="hex")
            for i in range(2):
                nc.vector.tensor_add(hexs[:, i, :ksz], octs[:, 2 * i, :ksz],
                                     octs[:, 2 * i + 1, :ksz])
            top = attn2.tile([128, 512], BF16, tag="top")
            nc.vector.tensor_add(top[:, :ksz], hexs[:, 0, :ksz],
                                 hexs[:, 1, :ksz])
            x0p = psum1.tile([128, 512], F32, tag="ps_acc")
            ssum = ssum_t[0:1, :]
            for m in range(32):
                eSm = expS_h[m // 8][:, m % 8, :ksz]
                nc.tensor.matmul(x0p[:, :ksz], x1t_s[:, m, :], eSm,
                                 start=(m == 0), stop=(m == 31))
            nc.tensor.matmul(ssum[:, :ksz], ones_s, top[:, :ksz],
                             start=True, stop=True)
            sinv = small.tile([1, 512], F32, tag="sinv")
            nc.vector.reciprocal(sinv[:, :ksz], ssum[:, :ksz])
            # two conv0 in0-half partial groups per k-tile fill the PE slack
            emit_in0_partial(2 * kt)
            emit_in0_partial(2 * kt + 1)
            if kt < 4:
                # broadcast 1/colsum to all partitions via a DRAM roundtrip
                # (latency hidden mid-attention)
                sinv_d = dram.tile([1, 512], F32, tag="sinv_d")
                nc.sync.dma_start(out=sinv_d[:, :ksz], in_=sinv[:, :ksz])
                sinvb = small.tile([128, 512], F32, tag="sinvb")
                nc.sync.dma_start(
                    out=sinvb[:, :ksz],
                    in_=sinv_d[:, :ksz].partition_broadcast(128)[:, 0, :])
                nc.vector.tensor_mul(x0n_s[:, k0:k0 + ksz], x0p[:, :ksz],
                                     sinvb[:, :ksz])
            else:
                # last tile feeds the serial attention->conv transition:
                # broadcast via a tiny fp32 ones-matmul instead (no DMA
                # latency on the critical path)
                bcast = psum1.tile([128, 512], F32, tag="ps_cv",
                                   name="sinv_bcast")
                nc.tensor.matmul(bcast[:, :ksz], ones_row, sinv[:, :ksz],
                                 start=True, stop=True)
                sinvb = small.tile([128, 512], F32, tag="sinvb")
                nc.vector.tensor_copy(sinvb[:, :ksz], bcast[:, :ksz])
                nc.vector.tensor_mul(x0n_s[:, k0:k0 + ksz], x0p[:, :ksz],
                                     sinvb[:, :ksz])

        # ---- out-projection + bn0 + gelu -> x0' (fp8) into ybuf ----
        for kt, (k0, ksz) in enumerate(K_TILES):
            nr = ksz // W  # rows in this k-tile
            for oc in range(2):
                po = psum.tile([128, 2, 512], F32, tag="ps_S")
                nc.tensor.matmul(po[:, 0, :ksz],
                                 wo_s[:, oc * 128:(oc + 1) * 128],
                                 x0n_s[:, k0:k0 + ksz],
                                 start=True, stop=True)
                nc.scalar.activation(
                    ybuf[:, oc, 1 + kt * 8:1 + kt * 8 + nr, 1:65],
                    po[:, 0, :ksz].rearrange("p (r w) -> p r w", w=W),
                    AF.Gelu, bias=bo_s[:, oc:oc + 1], scale=ao_s[:, oc:oc + 1])

        # ---- conv1 weights (loaded during attention; fp8 hi/lo) ----
        w1all_s = persist.tile([128, 36, C], F8, tag="w1all")
        nc.sync.dma_start(
            out=w1all_s,
            in_=w1all_d.rearrange("s t (a p) o -> p (s t a) o", a=2))
        w1h_s = w1all_s[:, 0:18]
        w1l_s = w1all_s[:, 18:36]

        # ---- conv0: x0'-half naive fp8 DoubleRow on top of the re-injected
        # in0-half partial (identity matmul opens the accumulation) ----
        for ri, (r0, nr) in enumerate(ROW_TILES):
            for oc in range(2):
                pc = psum.tile([128, 2, 512], F32, tag="ps_S")
                pcv = pc[:, 0, :nr * W].rearrange("p (r w) -> p r w", w=W)
                nc.tensor.matmul(pc[:, 0, :nr * W], id_s,
                                 inpart[ri * 2 + oc][:, :nr * W],
                                 start=True, stop=False)
                for t9 in range(9):
                    dh, dw = divmod(t9, 3)
                    nc.tensor.matmul(
                        pcv,
                        w0x_s[:, t9 * 2:t9 * 2 + 2, oc * 128:(oc + 1) * 128],
                        ybuf[:, :, r0 + dh - 1:r0 + dh - 1 + nr, dw:dw + W],
                        start=False, stop=(t9 == 8),
                        perf_mode=DR)
                nc.scalar.activation(
                    c0f[:, oc, r0 - 1:r0 - 1 + nr, :], pcv,
                    AF.Gelu, bias=b0_s[:, oc:oc + 1], scale=a0_s[:, oc:oc + 1])
                # hi/lo split of c0 for conv1's 3-term product (DVE)
                nc.vector.tensor_copy(c0h[:, oc, r0:r0 + nr, 1:65],
                                      c0f[:, oc, r0 - 1:r0 - 1 + nr, :])
                nc.vector.tensor_sub(c0l[:, oc, r0:r0 + nr, 1:65],
                                     c0f[:, oc, r0 - 1:r0 - 1 + nr, :],
                                     c0h[:, oc, r0:r0 + nr, 1:65])

        # ---- conv1: 256 -> 256, 3-term DoubleRow fp8, bn + gelu,
        #      + x0' residual, row-max; per-row-tile output DMA so only the
        #      small last tile sits on the kernel tail ----
        for r0, nr in ROW_TILES:
            for oc in range(2):
                pc = psum.tile([128, 2, 512], F32, tag="ps_S")
                pcv = pc[:, 0, :nr * W].rearrange("p (r w) -> p r w", w=W)
                terms1 = [(w1h_s, c0h), (w1h_s, c0l), (w1l_s, c0h)]
                i_mm, n_mm = 0, 9 * len(terms1)
                for w_s, x_s in terms1:
                    for t9 in range(9):
                        dh, dw = divmod(t9, 3)
                        nc.tensor.matmul(
                            pcv,
                            w_s[:, t9 * 2:t9 * 2 + 2, oc * 128:(oc + 1) * 128],
                            x_s[:, :, r0 + dh - 1:r0 + dh - 1 + nr, dw:dw + W],
                            start=(i_mm == 0), stop=(i_mm == n_mm - 1),
                            perf_mode=DR)
                        i_mm += 1
                tmp = small.tile([128, 512], F32, tag="scratch")
                nc.scalar.activation(tmp[:, :nr * W], pc[:, 0, :nr * W], AF.Gelu,
                                     bias=b1_s[:, oc:oc + 1],
                                     scale=a1_s[:, oc:oc + 1])
                res = small.tile([128, 512], F32, tag="scratch")
                nc.vector.tensor_add(
                    res[:, :nr * W].rearrange("p (r w) -> p r w", w=W),
                    tmp[:, :nr * W].rearrange("p (r w) -> p r w", w=W),
                    ybuf[:, oc, r0:r0 + nr, 1:65])
                outr = small.tile([128, 8], F32, tag="outr")
                nc.vector.reduce_max(
                    outr[:, :nr],
                    res[:, :nr * W].rearrange("p (r w) -> p r w", w=W),
                    axis=AX.X)
                # alternate HWDGE queues so the two last-tile DMAs overlap
                eng = nc.sync if oc == 0 else nc.scalar
                eng.dma_start(
                    out=out[oc * 128:(oc + 1) * 128, r0 - 1:r0 - 1 + nr],
                    in_=outr[:, :nr])

    nc.compile()
    return nc


def _prep_maps(inputs):
    """Host-side input prep: slicing, transposes, BN folding, fp8 splits."""
    f = np.float32
    in0 = np.ascontiguousarray(np.asarray(inputs["inputs_0"], f).reshape(B, C, N))
    in1 = np.ascontiguousarray(np.asarray(inputs["inputs_1"], f).reshape(B, C, N))
    g = {k: np.asarray(v, f) for k, v in inputs.items()}

    def fold(gm, bt, m, v, conv_b):
        a = (gm / np.sqrt(v + EPS)).astype(f)
        return a, (bt - m * a + a * conv_b).astype(f)

    a_bn, b_bn = fold(g["bn0_g"], g["bn0_b"], g["bn0_m"], g["bn0_v"],
                      g["b_o"] + g["w_o"] @ g["b_q"])
    a0, b0 = fold(g["cb_bn0_g"], g["cb_bn0_b"], g["cb_bn0_m"], g["cb_bn0_v"],
                  g["cb_b0"])
    a1, b1 = fold(g["cb_bn1_g"], g["cb_bn1_b"], g["cb_bn1_m"], g["cb_bn1_v"],
                  g["cb_b1"])

    def wsplit(w):
        wh = w.astype(F8NP)
        wl = (w - wh.astype(f)).astype(F8NP)
        return wh, wl

    # conv weights as (tap, ci, o); x0-half naive fp8, in0-half + w1 hi/lo
    w0t = np.ascontiguousarray(
        g["cb_w0"].transpose(2, 3, 1, 0).reshape(9, 2 * C, C))
    w1t = np.ascontiguousarray(
        g["cb_w1"].transpose(2, 3, 1, 0).reshape(9, C, C))
    w0inh, w0inl = wsplit(w0t[:, C:, :])
    w1h, w1l = wsplit(w1t)

    shared = {
        "wq_t": np.ascontiguousarray(g["w_q"].T).astype(BF16NP),
        "wk_t": np.ascontiguousarray(g["w_k"].T).astype(BF16NP),
        "wv_t": np.ascontiguousarray(g["w_v"].T).astype(BF16NP),
        "wo_t": np.ascontiguousarray(g["w_o"].T).astype(BF16NP),
        "bv": np.ascontiguousarray(g["b_v"].reshape(Ch, 1)),
        "bias6": np.ascontiguousarray(
            np.stack([a_bn, b_bn, a0, b0, a1, b1]).reshape(6, C, 1)),
        "w0all": np.ascontiguousarray(
            np.stack([w0t[:, :C, :].astype(F8NP), w0inh, w0inl])),
        "w1all": np.ascontiguousarray(np.stack([w1h, w1l])),
        "idm": np.eye(128, dtype=BF16NP),
    }
    maps = []
    for b in range(B):
        in0b16 = in0[b].astype(BF16NP)
        for half in range(2):
            w0r = 0 if half == 0 else 30
            sl = slice(w0r * W, (w0r + ROWS) * W)
            in0w_f32 = in0[b][:, sl].reshape(C, ROWS, W)
            in0h = np.zeros((C, ROWS, 66), F8NP)
            in0l = np.zeros((C, ROWS, 66), F8NP)
            in0h[:, :, 1:65] = in0w_f32.astype(F8NP)
            in0l[:, :, 1:65] = (
                in0w_f32 - in0h[:, :, 1:65].astype(f)).astype(F8NP)
            maps.append({
                "in0b": in0b16,
                "in0h": in0h.reshape(C, ROWS * 66),
                "in0l": in0l.reshape(C, ROWS * 66),
                "in1b": np.ascontiguousarray(in1[b][:, sl]).astype(BF16NP),
                **shared,
            })
    return maps


def kernel(**inputs):
    if "nc" not in _CACHED:
        _CACHED["nc"] = build_program()
    nc = _CACHED["nc"]
    maps = _prep_maps(inputs)
    res = run_bass_kernel_spmd(nc, maps, core_ids=list(range(8)))
    out = np.zeros((B, C), np.float32)
    for b in range(B):
        top = res.results[2 * b]["out"][:, 0:32].max(axis=1)
        bot = res.results[2 * b + 1]["out"][:, 2:34].max(axis=1)
        out[b] = np.maximum(out[b], np.maximum(top, bot))
    return out


# revision 39
# speedup vs baseline: 1.0218x; 1.0051x over previous
"""Trainium2 Bass kernel for nn_CFAConv (cross-feature attention + conv block).

Self-contained: takes full unsharded inputs, shards (batch, image-half) across
8 NeuronCores, runs one SPMD Bass/Tile NEFF, and combines partial results on
the host.

Math (validated against the jax reference in numpy):
  x1 = w_q@in0 + b_q ; x2 = w_k@in0 + b_k ; x3 = w_v@in1 + b_v  (1x1 convs)
  aff = softmax_j(x2^T x3) ; x0 = x1 @ aff
  x0' = gelu(bn0(w_o@x0 + b_o))
  y = gelu(bn(conv3x3(concat(x0', in0)))) ; y = gelu(bn(conv3x3(y)))
  out = max_spatial(y + x0')
On-device simplifications:
  - softmax over j is invariant to per-column shifts => b_k drops entirely
  - x2^T(x3 + b_v) = x2^T x3 + (x2^T b_v) 1^T    => fold b_v into x3
  - (x1 + b_q 1^T) @ aff = x1@aff + b_q 1^T (aff columns sum to 1)
    => fold w_o@b_q into the out-projection bias (host-side)
  - eval-mode BN folds to per-channel scale/bias, fused into the gelu ACT op
  - softmax normalization deferred past the x1@exp(S) matmul (divide x0 by
    column sums); sums via a 5-level bf16 DVE pre-sum tree + one ones-matmul
  - no max-subtraction in softmax: |S| <= ~60 here; exp fits fp32 (max ~e88)
Precision: bf16 operands with fp32 PSUM accumulation for the attention path;
the two 3x3 convs run in fp8e4m3 with DoubleRow perf mode (2 contraction
tiles per pass at 0.5 cycles/row):
  - conv0 x0'-half: weights + acts naive fp8 (x0' is small vs in0 => cheap)
  - conv0 in0-half: weights hi+lo fp8 split, in0 hi+lo fp8 split (host-side),
    3-term product (Wh Xh + Wh Xl + Wl Xh)
  - conv1: weights hi+lo (host), c0 hi+lo split on DVE, 3-term
  (numpy bit-model: 1.3e-2 final rel err vs the 2e-2 budget)
Sharding: 8 cores = (4 batches) x (top/bottom image half). Each core computes
a 34-row window (32 owned + halo) so the two 3x3 convs need no communication;
per-row maxes [256, 34] go to the host which slices owned rows and reduces.
"""

from contextlib import ExitStack

import ml_dtypes
import numpy as np

import concourse.bass as bass
import concourse.tile as tile
from concourse import bacc, mybir
from concourse.bass_utils import run_bass_kernel_spmd

B, C, H, W = 4, 256, 64, 64
Ch = C // 2          # 128
N = H * W            # 4096
ROWS = 34            # per-core row window (32 owned + 2 halo)
KW = ROWS * W        # 2176 window positions
EPS = 1e-5

F32 = mybir.dt.float32
BF16 = mybir.dt.bfloat16
F8 = mybir.dt.float8e4
AF = mybir.ActivationFunctionType
AX = mybir.AxisListType
DR = mybir.MatmulPerfMode.DoubleRow
BF16NP = ml_dtypes.bfloat16
F8NP = ml_dtypes.float8_e4m3

# attention k-tiles over the 2176-column window
K_TILES = [(0, 512), (512, 512), (1024, 512), (1536, 512), (2048, 128)]
# conv output row-tiles (local rows 1..34 of the 36-row padded buffer)
ROW_TILES = [(1, 8), (9, 8), (17, 8), (25, 8), (33, 2)]

_CACHED = {}


def build_program():
    nc = bacc.Bacc("TRN2", target_bir_lowering=False, debug=False)

    def din(name, shape, dt=F32):
        return nc.dram_tensor(name, shape, dt, kind="ExternalInput").ap()

    in0b_d = din("in0b", [C, N], BF16)
    in1b_d = din("in1b", [C, KW], BF16)
    # in0 conv window, fp8 hi/lo, pre-padded to 66 cols (zero side columns)
    in0h_d = din("in0h", [C, ROWS * 66], F8)
    in0l_d = din("in0l", [C, ROWS * 66], F8)
    wq_t = din("wq_t", [C, Ch], BF16)     # (c, i)
    wk_t = din("wk_t", [C, Ch], BF16)
    wv_t = din("wv_t", [C, Ch], BF16)
    wo_t = din("wo_t", [Ch, C], BF16)     # (i, o)
    bv = din("bv", [Ch, 1])
    bias6_d = din("bias6", [6, C, 1])     # ao, bo, a0, b0, a1, b1
    # conv0 weights: [x0-half naive, in0-half hi, in0-half lo] (tap, ci, o)
    w0all_d = din("w0all", [3, 9, C, C], F8)
    w1all_d = din("w1all", [2, 9, C, C], F8)  # [hi, lo]
    idm_d = din("idm", [128, 128], BF16)
    out = nc.dram_tensor("out", [C, ROWS], F32, kind="ExternalOutput").ap()

    with tile.TileContext(nc) as tc, ExitStack() as ctx:
        persist = ctx.enter_context(tc.tile_pool(name="persist", bufs=1))
        psum = ctx.enter_context(tc.tile_pool(name="psum", bufs=2, space="PSUM"))
        psum1 = ctx.enter_context(tc.tile_pool(name="psum1", bufs=2, space="PSUM"))
        small = ctx.enter_context(tc.tile_pool(name="small", bufs=3))

        # ---- inputs: bf16 quarters of in0 (one DMA each: per-slice deps
        # because Tile dependencies are whole-tile). Weights + a small first
        # slice of in0 go first so the first matmul starts ASAP. ----
        # DMA issue costs ~1.26us/queue: strictly alternate the projection
        # inputs across the SP and ACT HWDGE queues in consumption order
        in0ap = in0b_d.rearrange("(a p) n -> p a n", a=2)
        in0q0a = persist.tile([128, 2, 128], BF16, tag="in0q0a")
        nc.scalar.dma_start(out=in0q0a, in_=in0ap[:, :, 0:128])
        wk_s = persist.tile([128, 2, Ch], BF16, tag="wk")
        nc.sync.dma_start(out=wk_s, in_=wk_t.rearrange("(a p) n -> p a n", a=2))
        in0q0b = persist.tile([128, 2, 384], BF16, tag="in0q0b")
        nc.scalar.dma_start(out=in0q0b, in_=in0ap[:, :, 128:512])
        wq_s = persist.tile([128, 2, Ch], BF16, tag="wq")
        nc.sync.dma_start(out=wq_s, in_=wq_t.rearrange("(a p) n -> p a n", a=2))
        in0q = [None] + [persist.tile([128, 2, 512], BF16, tag=f"in0q{q}",
                                      name=f"in0q{q}") for q in range(1, 8)]
        in1q = [persist.tile([128, 2, 1152], BF16, tag=f"in1q{q}",
                             name=f"in1q{q}") for q in range(2)]
        for q in range(1, 8):
            eng = nc.scalar if q % 2 == 0 else nc.sync
            eng.dma_start(out=in0q[q], in_=in0ap[:, :, q * 512:(q + 1) * 512])
        nc.sync.dma_start(
            out=in1q[0][:, :, :1024],
            in_=in1b_d.rearrange("(a p) n -> p a n", a=2)[:, :, 0:1024])
        nc.scalar.dma_start(
            out=in1q[1],
            in_=in1b_d.rearrange("(a p) n -> p a n", a=2)[:, :, 1024:KW])
        wv_s = persist.tile([128, 2, Ch], BF16, tag="wv")
        nc.scalar.dma_start(out=wv_s,
                            in_=wv_t.rearrange("(a p) n -> p a n", a=2))
        bv_s = persist.tile([128, 1], F32, tag="bv")
        nc.scalar.dma_start(out=bv_s, in_=bv)
        wo_s = persist.tile([128, C], BF16, tag="wo")
        nc.sync.dma_start(out=wo_s, in_=wo_t)
        bias_s = persist.tile([128, 12], F32, tag="bias6")
        nc.sync.dma_start(out=bias_s,
                          in_=bias6_d.rearrange("t (a p) o -> p (t a o)", a=2))
        ao_s, bo_s = bias_s[:, 0:2], bias_s[:, 2:4]
        a0_s, b0_s = bias_s[:, 4:6], bias_s[:, 6:8]
        a1_s, b1_s = bias_s[:, 8:10], bias_s[:, 10:12]
        ones_s = persist.tile([128, 1], BF16, tag="ones")
        nc.vector.memset(ones_s, 1.0)
        ones_row = persist.tile([1, 128], F32, tag="ones_row")
        nc.vector.memset(ones_row, 1.0)
        id_s = persist.tile([128, 128], BF16, tag="idm")
        nc.sync.dma_start(out=id_s, in_=idm_d)

        # ---- projections: x2 [ch, N], x1T [j, i], x3 [ch, KW] (all bf16) --
        x2_s = persist.tile([128, N], BF16, tag="x2")
        x1t_s = persist.tile([128, 32, Ch], BF16, tag="x1t")
        x3_s = persist.tile([128, KW], BF16, tag="x3")

        def jc_slices(jc, lo, hi):
            """moving-operand slices [lo:hi) of in0 quarter jc (jc 0 is split
            into a 128-col head so the first matmul follows a small DMA)."""
            if jc > 0:
                return [(in0q[jc], lo, hi, lo)]
            out = []
            if lo < 128:
                out.append((in0q0a, lo, min(hi, 128), lo))
            if hi > 128:
                out.append((in0q0b, max(lo, 128) - 128, hi - 128,
                            max(lo, 128)))
            return out

        for jc in range(8):
            # x2 chunk: one accumulation group in one PSUM bank
            ps2 = psum.tile([128, 2, 512], F32, tag="ps_S")
            mms = [(t, a, b, o, cc) for cc in range(2)
                   for (t, a, b, o) in jc_slices(jc, 0, 512)]
            for i, (t, a, b, o, cc) in enumerate(mms):
                nc.tensor.matmul(ps2[:, 0, o:o + b - a], wk_s[:, cc, :],
                                 t[:, cc, a:b],
                                 start=(i == 0), stop=(i == len(mms) - 1))
            nc.vector.tensor_copy(x2_s[:, jc * 512:(jc + 1) * 512], ps2[:, 0, :])
            # x1t: all 4 j-subchunks in ONE group/bank, evicted in one ACT copy
            ps1 = psum1.tile([128, 512], F32, tag="ps_acc")
            mms = [(js, t, a, b, cc) for js in range(4) for cc in range(2)
                   for (t, a, b, _) in jc_slices(jc, js * 128, (js + 1) * 128)]
            for i, (js, t, a, b, cc) in enumerate(mms):
                nc.tensor.matmul(
                    ps1[:, js * 128:js * 128 + Ch],
                    t[:, cc, a:b], wq_s[:, cc, :],
                    start=(i == 0), stop=(i == len(mms) - 1))
            # x1t eviction: DVE while ACT's queue is still dispatching the
            # early input DMAs (~6.3us), ACT once it has drained
            if jc < 5:
                nc.vector.tensor_copy(
                    x1t_s[:, jc * 4:(jc + 1) * 4, :],
                    ps1.rearrange("p (a c) -> p a c", c=Ch))
            else:
                nc.scalar.activation(
                    x1t_s[:, jc * 4:(jc + 1) * 4, :],
                    ps1.rearrange("p (a c) -> p a c", c=Ch), AF.Copy)

        for k0, ksz in K_TILES:
            iq, off = (0, k0) if k0 < 1024 else (1, k0 - 1024)
            ps3 = psum.tile([128, 2, 512], F32, tag="ps_S")
            for cc in range(2):
                nc.tensor.matmul(ps3[:, 0, :ksz], wv_s[:, cc, :],
                                 in1q[iq][:, cc, off:off + ksz],
                                 start=(cc == 0), stop=(cc == 1))
            # x3 = psum + b_v : folds the v-bias into the affinity logits
            nc.vector.tensor_scalar_add(x3_s[:, k0:k0 + ksz], ps3[:, 0, :ksz],
                                        bv_s)

        # ---- conv buffers (fp8, padded 36x66 with zero ring) ----
        convbuf = ctx.enter_context(tc.tile_pool(name="convbuf", bufs=1))
        ybuf = convbuf.tile([128, 2, 36, 66], F8, tag="ybuf")   # x0' chunks
        in0h_s = convbuf.tile([128, 2, 36, 66], F8, tag="in0h")
        in0l_s = convbuf.tile([128, 2, 36, 66], F8, tag="in0l")
        c0h = convbuf.tile([128, 2, 36, 66], F8, tag="c0h")
        c0l = convbuf.tile([128, 2, 36, 66], F8, tag="c0l")
        c0f = convbuf.tile([128, 2, ROWS, W], BF16, tag="c0f")
        for tl in (ybuf, in0h_s, in0l_s, c0h, c0l):
            # zero the pad ring on the otherwise-idle Pool engine (write-only
            # memset; reading uninitialized SBUF can produce NaNs)
            nc.gpsimd.memset(tl[:, :, 0, :], 0.0)
            nc.gpsimd.memset(tl[:, :, 35, :], 0.0)
            if tl is in0h_s or tl is in0l_s:
                continue  # side columns arrive zero-padded via the DMA
            nc.gpsimd.memset(tl[:, :, 1:35, 0:1], 0.0)
            nc.gpsimd.memset(tl[:, :, 1:35, 65:66], 0.0)
        # in0 conv window ships as fp8 hi/lo straight into the padded tiles
        # (host pre-pads the 66-col side ring so the DMA stays 3-dim)
        nc.sync.dma_start(
            out=in0h_s[:, :, 1:35, :],
            in_=in0h_d.rearrange("(a p) n -> p a n", a=2))
        nc.sync.dma_start(
            out=in0l_s[:, :, 1:35, :],
            in_=in0l_d.rearrange("(a p) n -> p a n", a=2))

        # ---- conv0 weights (early: the in0-half partial sums run inside the
        # attention phase to fill PE slack while ACT grinds the exps) ----
        w0all_s = persist.tile([128, 54, C], F8, tag="w0all")
        nc.sync.dma_start(
            out=w0all_s,
            in_=w0all_d.rearrange("s t (a p) o -> p (s t a) o", a=2))
        w0x_s = w0all_s[:, 0:18]
        w0inh_s = w0all_s[:, 18:36]
        w0inl_s = w0all_s[:, 36:54]
        inpart = [persist.tile([128, 512], BF16, tag=f"inpart{g}",
                               name=f"inpart{g}") for g in range(10)]
        terms_in0 = [(w0inh_s, in0h_s), (w0inh_s, in0l_s), (w0inl_s, in0h_s)]

        def emit_in0_partial(g):
            """27 DoubleRow passes of conv0's in0-half for group g=(rt,oc),
            evicted to SBUF bf16 for later re-injection."""
            (r0, nr), oc = ROW_TILES[g // 2], g % 2
            ps = psum1.tile([128, 512], F32, tag="ps_cv", name=f"cv{g}")
            pcv = ps[:, :nr * W].rearrange("p (r w) -> p r w", w=W)
            i_mm, n_mm = 0, 27
            for w_s, x_s in terms_in0:
                for t9 in range(9):
                    dh, dw = divmod(t9, 3)
                    nc.tensor.matmul(
                        pcv,
                        w_s[:, t9 * 2:t9 * 2 + 2, oc * 128:(oc + 1) * 128],
                        x_s[:, :, r0 + dh - 1:r0 + dh - 1 + nr, dw:dw + W],
                        start=(i_mm == 0), stop=(i_mm == n_mm - 1),
                        perf_mode=DR)
                    i_mm += 1
            nc.vector.tensor_copy(inpart[g][:, :nr * W], ps[:, :nr * W])

        # ---- attention: S = x2^T x3, exp, x0 = x1 @ exp, sums, normalize ----
        attn = ctx.enter_context(tc.tile_pool(name="attn", bufs=4))
        attn2 = ctx.enter_context(tc.tile_pool(name="attn2", bufs=2))
        dram = ctx.enter_context(tc.tile_pool(name="dram", bufs=5, space="DRAM"))
        x0n_s = persist.tile([128, KW], BF16, tag="x0n")
        for kt, (k0, ksz) in enumerate(K_TILES):
            # four quarter-tiles under one bufs=4 tag: stage-2 consumes a
            # quarter while later quarters' exps still run, and the next
            # k-tile's exps begin as soon as a quarter is drained
            expS_h = [attn.tile([128, 8, 512], BF16, tag="expS",
                                name=f"expS{k0}_{h}") for h in range(4)]
            # ssum shares the ps_cv ring with the conv0 in0-half partials:
            # both have fast consumers so the rotation never stalls the PE
            ssum_t = psum1.tile([128, 512], F32, tag="ps_cv",
                                name=f"ssum{k0}")
            for mh in range(16):  # chunk pairs
                sp = psum.tile([128, 2, 512], F32, tag="ps_S")
                for i in range(2):
                    m = 2 * mh + i
                    nc.tensor.matmul(
                        sp[:, i, :ksz],
                        x2_s[:, m * 128:(m + 1) * 128],
                        x3_s[:, k0:k0 + ksz],
                        start=True, stop=True)
                eh = expS_h[mh // 4]
                nc.scalar.activation(
                    eh[:, (2 * mh) % 8:(2 * mh) % 8 + 2, :ksz],
                    sp[:, :, :ksz], AF.Exp)
            # 5-level bf16 pre-sum tree on DVE collapses the softmax
            # column-sum to ONE ones-matmul pass (sum error ~0.3%, only
            # scales the normalization)
            octs = attn2.tile([128, 4, 512], BF16, tag="oct")
            for h in range(4):
                pair = attn.tile([128, 4, 512], BF16, tag="pair",
                                 name=f"pair{k0}_{h}")
                for i in range(4):
                    nc.vector.tensor_add(pair[:, i, :ksz],
                                         expS_h[h][:, 2 * i, :ksz],
                                         expS_h[h][:, 2 * i + 1, :ksz])
                quad = attn.tile([128, 2, 512], BF16, tag="quad",
                                 name=f"quad{k0}_{h}")
                for i in range(2):
                    nc.vector.tensor_add(quad[:, i, :ksz],
                                         pair[:, 2 * i, :ksz],
                                         pair[:, 2 * i + 1, :ksz])
                nc.vector.tensor_add(octs[:, h, :ksz], quad[:, 0, :ksz],
                                     quad[:, 1, :ksz])
            hexs = attn2.tile([128, 2, 512], BF16, tag="hex")
            for i in range(2):
                nc.vector.tensor_add(hexs[:, i, :ksz], octs[:, 2 * i, :ksz],
                                     octs[:, 2 * i + 1, :ksz])
            top = attn2.tile([128, 512], BF16, tag="top")
            nc.vector.tensor_add(top[:, :ksz], hexs[:, 0, :ksz],
                                 hexs[:, 1, :ksz])
            x0p = psum1.tile([128, 512], F32, tag="ps_acc")
            ssum = ssum_t[0:1, :]
            for m in range(32):
                eSm = expS_h[m // 8][:, m % 8, :ksz]
                nc.tensor.matmul(x0p[:, :ksz], x1t_s[:, m, :], eSm,
                                 start=(m == 0), stop=(m == 31))
            nc.tensor.matmul(ssum[:, :ksz], ones_s, top[:, :ksz],
                             start=True, stop=True)
            sinv = small.tile([1, 512], F32, tag="sinv")
            nc.vector.reciprocal(sinv[:, :ksz], ssum[:, :ksz])
            # two conv0 in0-half partial groups per k-tile fill the PE slack
            emit_in0_partial(2 * kt)
            emit_in0_partial(2 * kt + 1)
            if kt < 4:
                # broadcast 1/colsum to all partitions via a DRAM roundtrip
                # (latency hidden mid-attention)
                sinv_d = dram.tile([1, 512], F32, tag="sinv_d")
                nc.sync.dma_start(out=sinv_d[:, :ksz], in_=sinv[:, :ksz])
                sinvb = small.tile([128, 512], F32, tag="sinvb")
                nc.sync.dma_start(
                    out=sinvb[:, :ksz],
                    in_=sinv_d[:, :ksz].partition_broadcast(128)[:, 0, :])
                nc.vector.tensor_mul(x0n_s[:, k0:k0 + ksz], x0p[:, :ksz],
                                     sinvb[:, :ksz])
            else:
                # last tile feeds the serial attention->conv transition:
                # broadcast via a tiny fp32 ones-matmul instead (no DMA
                # latency on the critical path)
                bcast = psum1.tile([128, 512], F32, tag="ps_cv",
                                   name="sinv_bcast")
                nc.tensor.matmul(bcast[:, :ksz], ones_row, sinv[:, :ksz],
                                 start=True, stop=True)
                sinvb = small.tile([128, 512], F32, tag="sinvb")
                nc.vector.tensor_copy(sinvb[:, :ksz], bcast[:, :ksz])
                nc.vector.tensor_mul(x0n_s[:, k0:k0 + ksz], x0p[:, :ksz],
                                     sinvb[:, :ksz])

        # ---- out-projection + bn0 + gelu -> x0' (fp8) into ybuf ----
        for kt, (k0, ksz) in enumerate(K_TILES):
            nr = ksz // W  # rows in this k-tile
            for oc in range(2):
                po = psum1.tile([128, 512], F32, name=f"po{kt}_{oc}",
                                tag="ps_acc" if oc == 0 else "ps_cv")
                nc.tensor.matmul(po[:, :ksz],
                                 wo_s[:, oc * 128:(oc + 1) * 128],
                                 x0n_s[:, k0:k0 + ksz],
                                 start=True, stop=True)
                nc.scalar.activation(
                    ybuf[:, oc, 1 + kt * 8:1 + kt * 8 + nr, 1:65],
                    po[:, :ksz].rearrange("p (r w) -> p r w", w=W),
                    AF.Gelu, bias=bo_s[:, oc:oc + 1], scale=ao_s[:, oc:oc + 1])

        # ---- conv1 weights (loaded during attention; fp8 hi/lo) ----
        w1all_s = persist.tile([128, 36, C], F8, tag="w1all")
        nc.sync.dma_start(
            out=w1all_s,
            in_=w1all_d.rearrange("s t (a p) o -> p (s t a) o", a=2))
        w1h_s = w1all_s[:, 0:18]
        w1l_s = w1all_s[:, 18:36]

        # ---- conv0: x0'-half naive fp8 DoubleRow on top of the re-injected
        # in0-half partial (identity matmul opens the accumulation) ----
        for ri, (r0, nr) in enumerate(ROW_TILES):
            for oc in range(2):
                pc = psum1.tile([128, 512], F32, name=f"c0ps{ri}_{oc}",
                                tag="ps_acc" if oc == 0 else "ps_cv")
                pcv = pc[:, :nr * W].rearrange("p (r w) -> p r w", w=W)
                nc.tensor.matmul(pc[:, :nr * W], id_s,
                                 inpart[ri * 2 + oc][:, :nr * W],
                                 start=True, stop=False)
                for t9 in range(9):
                    dh, dw = divmod(t9, 3)
                    nc.tensor.matmul(
                        pcv,
                        w0x_s[:, t9 * 2:t9 * 2 + 2, oc * 128:(oc + 1) * 128],
                        ybuf[:, :, r0 + dh - 1:r0 + dh - 1 + nr, dw:dw + W],
                        start=False, stop=(t9 == 8),
                        perf_mode=DR)
                nc.scalar.activation(
                    c0f[:, oc, r0 - 1:r0 - 1 + nr, :], pcv,
                    AF.Gelu, bias=b0_s[:, oc:oc + 1], scale=a0_s[:, oc:oc + 1])
                # hi/lo split of c0 for conv1's 3-term product (DVE)
                nc.vector.tensor_copy(c0h[:, oc, r0:r0 + nr, 1:65],
                                      c0f[:, oc, r0 - 1:r0 - 1 + nr, :])
                nc.vector.tensor_sub(c0l[:, oc, r0:r0 + nr, 1:65],
                                     c0f[:, oc, r0 - 1:r0 - 1 + nr, :],
                                     c0h[:, oc, r0:r0 + nr, 1:65])

        # ---- conv1: 256 -> 256, 3-term DoubleRow fp8, bn + gelu,
        #      + x0' residual, row-max; per-row-tile output DMA so only the
        #      small last tile sits on the kernel tail ----
        for ri, (r0, nr) in enumerate(ROW_TILES):
            for oc in range(2):
                pc = psum1.tile([128, 512], F32, name=f"c1ps{ri}_{oc}",
                                tag="ps_acc" if oc == 0 else "ps_cv")
                pcv = pc[:, :nr * W].rearrange("p (r w) -> p r w", w=W)
                terms1 = [(w1h_s, c0h), (w1h_s, c0l), (w1l_s, c0h)]
                i_mm, n_mm = 0, 9 * len(terms1)
                for w_s, x_s in terms1:
                    for t9 in range(9):
                        dh, dw = divmod(t9, 3)
                        nc.tensor.matmul(
                            pcv,
                            w_s[:, t9 * 2:t9 * 2 + 2, oc * 128:(oc + 1) * 128],
                            x_s[:, :, r0 + dh - 1:r0 + dh - 1 + nr, dw:dw + W],
                            start=(i_mm == 0), stop=(i_mm == n_mm - 1),
                            perf_mode=DR)
                        i_mm += 1
                tmp = small.tile([128, 512], F32, tag="scratch")
                nc.scalar.activation(tmp[:, :nr * W], pc[:, :nr * W], AF.Gelu,
                                     bias=b1_s[:, oc:oc + 1],
                                     scale=a1_s[:, oc:oc + 1])
                res = small.tile([128, 512], F32, tag="scratch")
                nc.vector.tensor_add(
                    res[:, :nr * W].rearrange("p (r w) -> p r w", w=W),
                    tmp[:, :nr * W].rearrange("p (r w) -> p r w", w=W),
                    ybuf[:, oc, r0:r0 + nr, 1:65])
                outr = small.tile([128, 8], F32, tag="outr")
                nc.vector.reduce_max(
                    outr[:, :nr],
                    res[:, :nr * W].rearrange("p (r w) -> p r w", w=W),
                    axis=AX.X)
                # alternate HWDGE queues so the two last-tile DMAs overlap
                eng = nc.sync if oc == 0 else nc.scalar
                eng.dma_start(
                    out=out[oc * 128:(oc + 1) * 128, r0 - 1:r0 - 1 + nr],
                    in_=outr[:, :nr])

    nc.compile()
    return nc


def _prep_maps(inputs):
    """Host-side input prep: slicing, transposes, BN folding, fp8 splits."""
    f = np.float32
    in0 = np.ascontiguousarray(np.asarray(inputs["inputs_0"], f).reshape(B, C, N))
    in1 = np.ascontiguousarray(np.asarray(inputs["inputs_1"], f).reshape(B, C, N))
    g = {k: np.asarray(v, f) for k, v in inputs.items()}

    def fold(gm, bt, m, v, conv_b):
        a = (gm / np.sqrt(v + EPS)).astype(f)
        return a, (bt - m * a + a * conv_b).astype(f)

    a_bn, b_bn = fold(g["bn0_g"], g["bn0_b"], g["bn0_m"], g["bn0_v"],
                      g["b_o"] + g["w_o"] @ g["b_q"])
    a0, b0 = fold(g["cb_bn0_g"], g["cb_bn0_b"], g["cb_bn0_m"], g["cb_bn0_v"],
                  g["cb_b0"])
    a1, b1 = fold(g["cb_bn1_g"], g["cb_bn1_b"], g["cb_bn1_m"], g["cb_bn1_v"],
                  g["cb_b1"])

    def wsplit(w):
        wh = w.astype(F8NP)
        wl = (w - wh.astype(f)).astype(F8NP)
        return wh, wl

    # conv weights as (tap, ci, o); x0-half naive fp8, in0-half + w1 hi/lo
    w0t = np.ascontiguousarray(
        g["cb_w0"].transpose(2, 3, 1, 0).reshape(9, 2 * C, C))
    w1t = np.ascontiguousarray(
        g["cb_w1"].transpose(2, 3, 1, 0).reshape(9, C, C))
    w0inh, w0inl = wsplit(w0t[:, C:, :])
    w1h, w1l = wsplit(w1t)

    shared = {
        "wq_t": np.ascontiguousarray(g["w_q"].T).astype(BF16NP),
        "wk_t": np.ascontiguousarray(g["w_k"].T).astype(BF16NP),
        "wv_t": np.ascontiguousarray(g["w_v"].T).astype(BF16NP),
        "wo_t": np.ascontiguousarray(g["w_o"].T).astype(BF16NP),
        "bv": np.ascontiguousarray(g["b_v"].reshape(Ch, 1)),
        "bias6": np.ascontiguousarray(
            np.stack([a_bn, b_bn, a0, b0, a1, b1]).reshape(6, C, 1)),
        "w0all": np.ascontiguousarray(
            np.stack([w0t[:, :C, :].astype(F8NP), w0inh, w0inl])),
        "w1all": np.ascontiguousarray(np.stack([w1h, w1l])),
        "idm": np.eye(128, dtype=BF16NP),
    }
    maps = []
    for b in range(B):
        in0b16 = in0[b].astype(BF16NP)
        for half in range(2):
            w0r = 0 if half == 0 else 30
            sl = slice(w0r * W, (w0r + ROWS) * W)
            in0w_f32 = in0[b][:, sl].reshape(C, ROWS, W)
            in0h = np.zeros((C, ROWS, 66), F8NP)
            in0l = np.zeros((C, ROWS, 66), F8NP)
            in0h[:, :, 1:65] = in0w_f32.astype(F8NP)
            in0l[:, :, 1:65] = (
                in0w_f32 - in0h[:, :, 1:65].astype(f)).astype(F8NP)
            maps.append({
                "in0b": in0b16,
                "in0h": in0h.reshape(C, ROWS * 66),
                "in0l": in0l.reshape(C, ROWS * 66),
                "in1b": np.ascontiguousarray(in1[b][:, sl]).astype(BF16NP),
                **shared,
            })
    return maps


def kernel(**inputs):
    if "nc" not in _CACHED:
        _CACHED["nc"] = build_program()
    nc = _CACHED["nc"]
    maps = _prep_maps(inputs)
    res = run_bass_kernel_spmd(nc, maps, core_ids=list(range(8)))
    out = np.zeros((B, C), np.float32)
    for b in range(B):
        top = res.results[2 * b]["out"][:, 0:32].max(axis=1)
        bot = res.results[2 * b + 1]["out"][:, 2:34].max(axis=1)
        out[b] = np.maximum(out[b], np.maximum(top, bot))
    return out
